# revision 52
# baseline (speedup 1.0000x reference)
"""GTLayer (relational graph transformer layer) on 8 Trainium2 NeuronCores.

v2 strategy (see kernel_v1_baseline.py for the original):
- Nodes partitioned across 8 cores in graph-aligned contiguous slices;
  edges live with the core owning dst. Global gather tables laid out in 4
  row-pieces so allgathers can be fired piecewise and overlapped.
- fp16 data lane for relconv (hn table, gathered rows, W_rel/W_loop, S
  sums), bf16 lane for attention (kv table, one-hots, V*exp) — exp(score)
  products can exceed fp16 range. PSUM accumulation is always f32.
- RelConv: edges per (dst-window 256, src-half, rel-PAIR); S psum tiles are
  [128, 512] covering two relations (one-hot column = dl + 256*parity), so
  all 9 relations fit one PSUM residency and gather call regions merge to
  (window, half).
- One-hots built with tensor_scalar(is_equal, scalar1=key column) against
  an iota ramp (2x DVE mode), not tensor_tensor broadcasts.
- Attention: sw-blocks of 4 subwindows share gather calls; chunks
  processed in batches of <=4 with batched vector ops.
- Scalar engine does psum->sbuf copies (Copy), Relu/Exp/Square, arranged
  so activation tables almost never reload. LN1/LN2 sqrt is batched
  (columns collected across subwindows, one Sqrt instruction each).
- Biases applied as K=1 matmuls (ones-row lhsT) accumulating into PSUM.
- LN1 affine folded into graphNorm2's alpha/beta algebra.
"""
import os
import sys
import types
import numpy as np

NCORES = 8
N_NODES = 100000
N_EDGES = 600000
D = 128
REL = 9
NPAIR = 5
NG = 64
HEADS = 8
DH = 16
WIN = 256          # relconv dst window
GMAX = 16          # max graphs per core
PIECES = 4         # allgather pieces; also gather base regions (idx >= 0 always)


def _ensure_hooks():
    if "antenv.axon_hooks" not in sys.modules:
        hooks = types.ModuleType("antenv.axon_hooks")
        h = [None]
        hooks.set_axon_ntff_profile_hook = lambda v: h.__setitem__(0, v)
        hooks.get_axon_ntff_profile_hook = lambda: h[0]
        sys.modules["antenv.axon_hooks"] = hooks
        try:
            from trn_agent_boot.trn_boot import _ntff_profile_via_ctypes
            hooks.set_axon_ntff_profile_hook(
                _ntff_profile_via_ctypes("/opt/axon/libaxon_pjrt.so"))
        except Exception:
            pass


# ----------------------------------------------------------------------------
# Host preprocessing
# ----------------------------------------------------------------------------

def _pack_idx16(idx):
    """int16 index array -> [128, n/16] wrapped+replicated layout."""
    n = len(idx)
    assert n % 16 == 0
    blk = idx.reshape(n // 16, 16).T
    return np.tile(blk, (8, 1)).astype(np.int16)


def _layout_slots(order_edges, idx_vals, key_vals, n_chunks):
    """Place edges into n_chunks*128 slots (full 128 per chunk), pads get
    idx 0, key -1. Returns (idx int32, key f32)."""
    tot = n_chunks * 128
    idx = np.zeros(tot, np.int32)
    key = np.full(tot, -1.0, np.float32)
    ne = len(order_edges)
    assert ne <= tot, (ne, n_chunks)
    idx[:ne] = idx_vals[order_edges]
    key[:ne] = key_vals[order_edges]
    return idx, key


def preprocess(inputs):
    import ml_dtypes
    bf16 = ml_dtypes.bfloat16
    h = np.asarray(inputs['h'], np.float32)
    src = np.asarray(inputs['src']).astype(np.int64)
    dst = np.asarray(inputs['dst']).astype(np.int64)
    et = np.asarray(inputs['etypes']).astype(np.int64)
    seg = np.asarray(inputs['seg']).astype(np.int64)

    # --- graph-aligned node partition ---
    gstart = np.searchsorted(seg, np.arange(NG + 1))
    bounds = [0]
    for c in range(1, NCORES):
        target = c * N_NODES / NCORES
        g = int(np.argmin(np.abs(gstart - target)))
        bounds.append(int(gstart[g]))
    bounds.append(N_NODES)
    n0 = np.array(bounds[:-1]); n1 = np.array(bounds[1:])
    sizes = n1 - n0
    ROUND = max(512, WIN * PIECES)
    P_NODES = int(np.ceil(sizes.max() / ROUND) * ROUND)
    RPP = P_NODES // PIECES
    NW = P_NODES // WIN
    NSW = NW * 2
    assert NSW % 4 == 0
    NSWB = NSW // 4
    NTAB = NCORES * P_NODES
    assert NCORES * RPP <= 32768  # per-piece base region fits int16 idx

    owner = np.searchsorted(n1, np.arange(N_NODES), side='right')
    rloc = np.arange(N_NODES) - n0[owner]
    piece = rloc // RPP
    gpos = piece * (NCORES * RPP) + owner * RPP + (rloc - piece * RPP)

    g0 = np.searchsorted(gstart, n0, side='right') - 1
    counts_g = np.diff(gstart).astype(np.float32)

    BASEQ = NCORES * RPP
    srcp = gpos[src]
    half = srcp // BASEQ          # source piece = gather base region (0..3)
    ecore = owner[dst]
    dst_off = dst - n0[ecore]
    w_e = dst_off // WIN
    pair_e = et // 2
    par_e = et % 2
    key512 = (dst_off % WIN + 256 * par_e).astype(np.float32)
    sw_e = dst_off // 128
    dl128 = (dst_off % 128).astype(np.float32)
    swb_e = sw_e // 4
    swin_e = sw_e % 4
    idx_rel = (srcp - half * BASEQ).astype(np.int32)
    assert idx_rel.min() >= 0 and idx_rel.max() < 32768

    # --- relconv chunk structure: uniform K_RC chunks per (w, pc, pair) ---
    NQ = NW * PIECES * NPAIR
    rkey = (w_e * PIECES + half) * NPAIR + pair_e
    rc_counts = np.zeros((NCORES, NQ), np.int64)
    for c in range(NCORES):
        rc_counts[c] = np.bincount(rkey[ecore == c], minlength=NQ)
    K_RC = int(np.ceil(rc_counts / 128.0).max())
    RC_CHUNKS = NQ * K_RC

    # --- attention chunk structure: uniform K_AT per (swb, pc, swin) ---
    NAQ = NSWB * PIECES * 4
    akey = (swb_e * PIECES + half) * 4 + swin_e
    at_counts = np.zeros((NCORES, NAQ), np.int64)
    for c in range(NCORES):
        at_counts[c] = np.bincount(akey[ecore == c], minlength=NAQ)
    K_AT = int(np.ceil(at_counts / 128.0).max())
    AT_CHUNKS = NAQ * K_AT

    # --- per-core data arrays ---
    in_maps = []
    for c in range(NCORES):
        m = np.nonzero(ecore == c)[0]
        # order by (group, srcp) for gather locality
        order = np.lexsort((srcp[m], rkey[m]))
        es = m[order]
        rk = rkey[m][order]
        run_s = np.searchsorted(rk, np.arange(NQ))
        run_e = np.searchsorted(rk, np.arange(NQ) + 1)
        rc_idx = np.zeros(RC_CHUNKS * 128, np.int32)
        rc_key = np.full(RC_CHUNKS * 128, -1.0, np.float32)
        for q in range(NQ):
            ii, kk = _layout_slots(es[run_s[q]:run_e[q]], idx_rel, key512, K_RC)
            rc_idx[q * K_RC * 128:(q + 1) * K_RC * 128] = ii
            rc_key[q * K_RC * 128:(q + 1) * K_RC * 128] = kk

        aorder = np.lexsort((srcp[m], akey[m]))
        aes = m[aorder]
        ak = akey[m][aorder]
        arun_s = np.searchsorted(ak, np.arange(NAQ))
        arun_e = np.searchsorted(ak, np.arange(NAQ) + 1)
        at_idx = np.zeros(AT_CHUNKS * 128, np.int32)
        at_key = np.full(AT_CHUNKS * 128, -1.0, np.float32)
        for q in range(NAQ):
            ii, kk = _layout_slots(aes[arun_s[q]:arun_e[q]], idx_rel, dl128, K_AT)
            at_idx[q * K_AT * 128:(q + 1) * K_AT * 128] = ii
            at_key[q * K_AT * 128:(q + 1) * K_AT * 128] = kk
        # swin-major key copy for batched per-swin one-hots:
        # global chunk (swb, pc, swin, k) -> swin-major (swb, swin, pc, k)
        at_key_sw = (at_key.reshape(NSWB, PIECES, 4, K_AT, 128)
                     .transpose(0, 2, 1, 3, 4).reshape(AT_CHUNKS * 128))

        hs = np.zeros((P_NODES, D), np.float32)
        hs[:sizes[c]] = h[n0[c]:n1[c]]
        segl = np.full(P_NODES, -1.0, np.float32)
        segl[:sizes[c]] = (seg[n0[c]:n1[c]] - g0[c]).astype(np.float32)
        ginc = np.zeros((GMAX, 1), np.float32)
        ng_c = int(seg[n1[c] - 1] - g0[c]) + 1
        assert ng_c <= GMAX
        ginc[:ng_c, 0] = 1.0 / counts_g[g0[c]:g0[c] + ng_c]

        im = {
            'h_slice': hs.astype(np.float16),
            'seg_col': segl.reshape(NSW, 128).T.copy(),
            'inv_cnt': ginc,
            'rc_idx': _pack_idx16(rc_idx.astype(np.int16)),
            'rc_key': rc_key.reshape(RC_CHUNKS, 128).T.copy(),
            'at_idx': _pack_idx16(at_idx.astype(np.int16)),
            'at_key_sw': at_key_sw.reshape(AT_CHUNKS, 128).T.copy(),
        }
        in_maps.append(im)

    # --- shared weights ---
    def A(x):
        return np.ascontiguousarray(np.asarray(x, np.float32))
    Wrel = np.concatenate([
        np.einsum('rb,bio->rio', A(inputs[f'{nm}_coeff']), A(inputs[f'{nm}_basis']))
        for nm in ('q', 'k', 'v')], axis=2)            # [9, 128, 384]
    Wrel[:, :, :D] *= 0.25  # fold score/sqrt(dh) into Q
    wloop = np.concatenate([A(inputs[f'{nm}_loop']) for nm in ('q', 'k', 'v')], 1)
    wloop[:, :D] *= 0.25
    bqkv = np.concatenate([A(inputs[f'{nm}_bias']) for nm in ('q', 'k', 'v')])
    bqkv[:D] *= 0.25
    ffn2p = np.zeros((D, 2 * D), np.float32)           # two K-chunks side by side
    ffn2p[:, :D] = A(inputs['ffn2_w'])[:D, :]
    ffn2p[:, D:] = A(inputs['ffn2_w'])[D:, :]
    # head-replication one-hots: hrep[(s', h), f] = 1 if s' == s and f // DH == h
    hrep = np.zeros((4 * HEADS, 4 * D), bf16)
    for s in range(4):
        for h in range(HEADS):
            hrep[s * HEADS + h, s * D + h * DH:s * D + (h + 1) * DH] = 1.0
    w_shared = {
        'w_rel': A(Wrel.reshape(REL * D, 3 * D)).astype(np.float16),
        'w_loop': wloop.astype(np.float16),
        'brow_qkv': bqkv.reshape(1, 3 * D).astype(np.float16),
        'o_w': A(inputs['o_w']).astype(bf16),
        'brow_o': A(inputs['o_b']).reshape(1, D).astype(bf16),
        'brow_o4': np.tile(A(inputs['o_b']).reshape(1, D), (1, 4)).astype(bf16),
        'hrep': hrep,
        'ffn1': A(inputs['ffn1_w']).astype(np.float16),
        'brow_f1': A(inputs['ffn1_b']).reshape(1, 2 * D).astype(np.float16),
        'ffn2': ffn2p.astype(np.float16),
        'brow_f2': A(inputs['ffn2_b']).reshape(1, D).astype(np.float16),
        'ln2_g': np.tile(A(inputs['ln2_g'])[None, :], (128, 1)),
        'ln2_b': np.tile(A(inputs['ln2_b'])[None, :], (128, 1)),
    }
    for nm in ('gn1', 'gn2'):
        w = A(inputs[f'{nm}_w']); b = A(inputs[f'{nm}_b']); ms = A(inputs[f'{nm}_ms'])
        w_shared[f'{nm}_w16'] = np.tile(w[None, :], (GMAX, 1))
        w_shared[f'{nm}_b16'] = np.tile(b[None, :], (GMAX, 1))
        w_shared[f'{nm}_ms16'] = np.tile(ms[None, :], (GMAX, 1))
        w_shared[f'{nm}_msfac16'] = np.tile((ms * (2 - ms))[None, :], (GMAX, 1))
    for nm in ('ln1_g', 'ln1_b'):
        w_shared[f'{nm}16'] = np.tile(A(inputs[nm])[None, :], (GMAX, 1))
    for im in in_maps:
        im.update(w_shared)

    static = dict(P_NODES=P_NODES, NW=NW, NSW=NSW, NSWB=NSWB, RPP=RPP,
                  NTAB=NTAB, K_RC=K_RC, K_AT=K_AT,
                  RC_CHUNKS=RC_CHUNKS, AT_CHUNKS=AT_CHUNKS)
    meta = dict(n0=n0, n1=n1, sizes=sizes)
    return static, in_maps, meta


# ----------------------------------------------------------------------------
# Bass program
# ----------------------------------------------------------------------------

_PROGRAM_CACHE = {}


def build_program(st):
    import concourse.bass as bass
    import concourse.bacc as bacc
    import concourse.mybir as mybir
    import concourse.tile as tile
    from concourse.tile import TileContext
    from concourse.masks import make_identity
    from bass_rust import add_dep_helper

    P_NODES = st['P_NODES']; NW = st['NW']; NSW = st['NSW']
    NSWB = st['NSWB']; RPP = st['RPP']; NTAB = st['NTAB']
    BASEQ = NCORES * RPP
    K_RC = st['K_RC']; K_AT = st['K_AT']
    RC_CHUNKS = st['RC_CHUNKS']; AT_CHUNKS = st['AT_CHUNKS']
    RC_TAKE = NPAIR * K_RC          # chunks per relconv gather call (w, pc)
    AT_TAKE = 4 * K_AT              # chunks per attention gather call (swb, pc)
    CH_SW = PIECES * K_AT           # chunks per subwindow
    f32 = mybir.dt.float32
    f16 = mybir.dt.float16
    b16 = mybir.dt.bfloat16
    i16 = mybir.dt.int16
    AO = mybir.AluOpType
    AF = mybir.ActivationFunctionType

    NQUEUE = int(os.environ.get('KERNEL_NQUEUE', '2'))
    nc = bacc.Bacc(num_swdge_queues=NQUEUE) if NQUEUE > 1 else bacc.Bacc()

    # --- I/O ---
    h_slice = nc.declare_dram_parameter('h_slice', [P_NODES, D], f16, isOutput=False)
    seg_col = nc.declare_dram_parameter('seg_col', [128, NSW], f32, isOutput=False)
    inv_cnt = nc.declare_dram_parameter('inv_cnt', [GMAX, 1], f32, isOutput=False)
    rc_idx = nc.declare_dram_parameter('rc_idx', [128, RC_CHUNKS * 8], i16, isOutput=False)
    rc_keyd = nc.declare_dram_parameter('rc_key', [128, RC_CHUNKS], f32, isOutput=False)
    at_idx = nc.declare_dram_parameter('at_idx', [128, AT_CHUNKS * 8], i16, isOutput=False)
    at_keyd = nc.declare_dram_parameter('at_key_sw', [128, AT_CHUNKS], f32, isOutput=False)
    hrep_d = nc.declare_dram_parameter('hrep', [4 * HEADS, 4 * D], b16, isOutput=False)
    brow_o4 = nc.declare_dram_parameter('brow_o4', [1, 4 * D], b16, isOutput=False)
    w_rel = nc.declare_dram_parameter('w_rel', [REL * D, 3 * D], f16, isOutput=False)
    w_loop = nc.declare_dram_parameter('w_loop', [D, 3 * D], f16, isOutput=False)
    brow_qkv = nc.declare_dram_parameter('brow_qkv', [1, 3 * D], f16, isOutput=False)
    o_w = nc.declare_dram_parameter('o_w', [D, D], b16, isOutput=False)
    brow_o = nc.declare_dram_parameter('brow_o', [1, D], b16, isOutput=False)
    ffn1 = nc.declare_dram_parameter('ffn1', [D, 2 * D], f16, isOutput=False)
    brow_f1 = nc.declare_dram_parameter('brow_f1', [1, 2 * D], f16, isOutput=False)
    ffn2 = nc.declare_dram_parameter('ffn2', [D, 2 * D], f16, isOutput=False)
    brow_f2 = nc.declare_dram_parameter('brow_f2', [1, D], f16, isOutput=False)
    cdecl = {}
    for nm in ('ln2_g', 'ln2_b'):
        cdecl[nm] = nc.declare_dram_parameter(nm, [128, D], f32, isOutput=False)
    for nm in ('gn1_w16', 'gn1_b16', 'gn1_ms16', 'gn1_msfac16',
               'gn2_w16', 'gn2_b16', 'gn2_ms16', 'gn2_msfac16',
               'ln1_g16', 'ln1_b16'):
        cdecl[nm] = nc.declare_dram_parameter(nm, [GMAX, D], f32, isOutput=False)
    out_sl = nc.declare_dram_parameter('out_slice', [P_NODES, D], f32, isOutput=True)

    # --- internal DRAM ---
    hn_local = nc.dram_tensor('hn_local', [P_NODES, D], f16)
    q_local = nc.dram_tensor('q_local', [P_NODES, D], b16)
    kv_local = nc.dram_tensor('kv_local', [P_NODES, 2 * D], b16)
    hn_full = nc.dram_tensor('hn_full', [NTAB, D], f16, addr_space='Shared')
    kv_full = nc.dram_tensor('kv_full', [NTAB, 2 * D], b16, addr_space='Shared')
    debug = os.environ.get('KERNEL_DEBUG') == '1'
    if debug:
        hn_dbg = nc.declare_dram_parameter('hn_dbg', [P_NODES, D], f16, isOutput=True)
        kv_dbg = nc.declare_dram_parameter('kv_dbg', [P_NODES, 2 * D], b16, isOutput=True)
        q_dbg = nc.declare_dram_parameter('q_dbg', [P_NODES, D], b16, isOutput=True)
        wv_dbg = nc.declare_dram_parameter('wv_dbg', [NSWB * 128, 4 * D], b16, isOutput=True)
        z_dbg = nc.declare_dram_parameter('z_dbg', [NSWB * 128, 4 * HEADS], f32, isOutput=True)
        hob_dbg = nc.declare_dram_parameter('hob_dbg', [P_NODES, D], f16, isOutput=True)
        attnt_dbg = nc.declare_dram_parameter('attnt_dbg', [NSWB * 128, 4 * D], b16, isOutput=True)
        hov_dbg = nc.declare_dram_parameter('hov_dbg', [P_NODES, D], f32, isOutput=True)
        zrt_dbg = nc.declare_dram_parameter('zrt_dbg', [NSWB * 32, 128], b16, isOutput=True)
        zrec_dbg = nc.declare_dram_parameter('zrec_dbg', [NSWB * 128, 4 * HEADS], b16, isOutput=True)

    with TileContext(nc) as tc:
        with tc.tile_pool(name='const', bufs=1) as cpool:
            iota = cpool.tile([128, 2 * WIN], f16)
            nc.gpsimd.iota(iota[:], pattern=[[1, 2 * WIN]], base=0,
                           channel_multiplier=0, allow_small_or_imprecise_dtypes=True)
            iota_b = cpool.tile([128, 128], b16)
            nc.gpsimd.iota(iota_b[:], pattern=[[1, 128]], base=0,
                           channel_multiplier=0, allow_small_or_imprecise_dtypes=True)
            ident_f = cpool.tile([128, 128], f16)
            make_identity(nc, ident_f[:])
            ident_b = cpool.tile([128, 128], b16)
            make_identity(nc, ident_b[:])
            ones_f = cpool.tile([1, 128], f16)
            nc.gpsimd.memset(ones_f[:], 1.0)
            ones_b = cpool.tile([1, 128], b16)
            nc.gpsimd.memset(ones_b[:], 1.0)

            segs = cpool.tile([128, NSW], f32)
            nc.sync.dma_start(out=segs[:], in_=seg_col[:])
            rck = cpool.tile([128, RC_CHUNKS], f32)
            nc.sync.dma_start(out=rck[:], in_=rc_keyd[:])
            atk = cpool.tile([128, AT_CHUNKS], f32)
            nc.sync.dma_start(out=atk[:], in_=at_keyd[:])

            wrel_sb = cpool.tile([128, REL * 3 * D], f16)
            for r in range(REL):
                nc.sync.dma_start(out=wrel_sb[:, r * 3 * D:(r + 1) * 3 * D],
                                  in_=w_rel[r * D:(r + 1) * D, :])
            wloop_sb = cpool.tile([128, 3 * D], f16)
            nc.sync.dma_start(out=wloop_sb[:], in_=w_loop[:])
            brqkv_sb = cpool.tile([1, 3 * D], f16)
            nc.sync.dma_start(out=brqkv_sb[:], in_=brow_qkv[:])
            ow_sb = cpool.tile([D, D], b16)
            nc.sync.dma_start(out=ow_sb[:], in_=o_w[:])
            bro_sb = cpool.tile([1, D], b16)
            nc.sync.dma_start(out=bro_sb[:], in_=brow_o[:])
            bro4_sb = cpool.tile([1, 4 * D], b16)
            nc.sync.dma_start(out=bro4_sb[:], in_=brow_o4[:])
            hrep_sb = cpool.tile([4 * HEADS, 4 * D], b16)
            nc.sync.dma_start(out=hrep_sb[:], in_=hrep_d[:])
            ffn1_sb = cpool.tile([D, 2 * D], f16)
            nc.sync.dma_start(out=ffn1_sb[:], in_=ffn1[:])
            brf1_sb = cpool.tile([1, 2 * D], f16)
            nc.sync.dma_start(out=brf1_sb[:], in_=brow_f1[:])
            ffn2_sb = cpool.tile([D, 2 * D], f16)
            nc.sync.dma_start(out=ffn2_sb[:], in_=ffn2[:])
            brf2_sb = cpool.tile([1, D], f16)
            nc.sync.dma_start(out=brf2_sb[:], in_=brow_f2[:])
            csb = {}
            for nm, dd in cdecl.items():
                t = cpool.tile(list(dd.shape), f32, tag=f'c_{nm}')
                nc.sync.dma_start(out=t[:], in_=dd[:])
                csb[nm] = t
            invc_sb = cpool.tile([GMAX, 1], f32)
            nc.sync.dma_start(out=invc_sb[:], in_=inv_cnt[:])

            # batched LN stat columns
            ln1_ns = cpool.tile([128, NSW], f32, tag='ln1_ns')
            ln1_sq = cpool.tile([128, NSW], f32, tag='ln1_sq')
            ln1_rsd = cpool.tile([128, NSW], f32, tag='ln1_rsd')
            ln1_nmr = cpool.tile([128, NSW], f32, tag='ln1_nmr')
            ln2_vr = cpool.tile([128, NSW], f32, tag='ln2_vr')
            ln2_rsd = cpool.tile([128, NSW], f32, tag='ln2_rsd')

            # persistent pools
            bpool_cm = tc.tile_pool(name='bpool', bufs=1)
            bpool = bpool_cm.__enter__()
            hobp_cm = tc.tile_pool(name='hobp', bufs=1)
            hobp = hobp_cm.__enter__()
            xc2p_cm = tc.tile_pool(name='xc2p', bufs=1)
            xc2p = xc2p_cm.__enter__()
            k4 = tc.tile_pool(name='p4keep', bufs=1)
            keep4 = k4.__enter__()
            Bt = []      # [128, GMAX] f16 one-hot per subwindow
            hobt = []    # [128, D] f16 attn-out per subwindow
            xc2t = []    # [128, D] f16 LN2-centered per subwindow

            # =========== phase 1: graphNorm1 ===========
            sc1 = nc.enter_named_scope('ph1_gn1', False)
            cc_hn = []
            with (
                tc.tile_pool(name='p1keep', bufs=1) as keep1,
                tc.tile_pool(name='p1sb', bufs=3) as sb1,
                tc.tile_pool(name='p1ps', bufs=1, space='PSUM') as ps1,
                tc.tile_pool(name='p1ps2', bufs=2, space='PSUM') as ps1b,
            ):
                sum_ps = ps1.tile([GMAX, D], f32, tag='sums')
                sq_ps = ps1.tile([GMAX, D], f32, tag='sqs')
                for s0 in range(0, NSW, 4):
                    hw = sb1.tile([128, 4 * D], f16, tag='h_in')
                    nc.sync.dma_start(
                        out=hw[:].rearrange('p (s d) -> p s d', d=D),
                        in_=h_slice[s0 * 128:(s0 + 4) * 128, :]
                            .rearrange('(s p) d -> p s d', p=128))
                    B4 = bpool.tile([128, 4 * GMAX], f16, tag=f'B_{s0 // 4}',
                                    name=f'B_{s0 // 4}')
                    nc.vector.tensor_tensor(
                        out=B4[:].rearrange('p (s g) -> p s g', g=GMAX),
                        in0=segs[:, s0:s0 + 4].rearrange('p (s o) -> p s o', o=1)
                            .to_broadcast([128, 4, GMAX]),
                        in1=iota[:, :GMAX].rearrange('p (o g) -> p o g', o=1)
                            .to_broadcast([128, 4, GMAX]),
                        op=AO.is_equal)
                    Bt.append(B4)
                    hsq = sb1.tile([128, 4 * D], f16, tag='hsq')
                    nc.scalar.activation(out=hsq[:], in_=hw[:], func=AF.Square)
                    for j in range(4):
                        s = s0 + j
                        nc.tensor.matmul(out=sum_ps[:],
                                         lhsT=B4[:, j * GMAX:(j + 1) * GMAX],
                                         rhs=hw[:, j * D:(j + 1) * D],
                                         start=(s == 0), stop=(s == NSW - 1))
                        nc.tensor.matmul(out=sq_ps[:],
                                         lhsT=B4[:, j * GMAX:(j + 1) * GMAX],
                                         rhs=hsq[:, j * D:(j + 1) * D],
                                         start=(s == 0), stop=(s == NSW - 1))
                # finalize -> alpha/beta [GMAX, D] f32
                mean = keep1.tile([GMAX, D], f32)
                nc.vector.tensor_tensor(out=mean[:], in0=sum_ps[:],
                                        in1=invc_sb[:].to_broadcast([GMAX, D]), op=AO.mult)
                ex2 = keep1.tile([GMAX, D], f32)
                nc.vector.tensor_tensor(out=ex2[:], in0=sq_ps[:],
                                        in1=invc_sb[:].to_broadcast([GMAX, D]), op=AO.mult)
                msq = keep1.tile([GMAX, D], f32)
                nc.vector.tensor_tensor(out=msq[:], in0=mean[:], in1=mean[:], op=AO.mult)
                nc.vector.tensor_tensor(out=msq[:], in0=msq[:], in1=csb['gn1_msfac16'][:], op=AO.mult)
                var = keep1.tile([GMAX, D], f32)
                nc.vector.tensor_tensor(out=var[:], in0=ex2[:], in1=msq[:], op=AO.subtract)
                nc.vector.tensor_scalar_add(out=var[:], in0=var[:], scalar1=1e-6)
                std = keep1.tile([GMAX, D], f32)
                nc.scalar.activation(out=std[:], in_=var[:], func=AF.Sqrt)
                rstd = keep1.tile([GMAX, D], f32)
                nc.vector.reciprocal(out=rstd[:], in_=std[:])
                alpha1 = keep1.tile([GMAX, D], f32)
                nc.vector.tensor_tensor(out=alpha1[:], in0=rstd[:], in1=csb['gn1_w16'][:], op=AO.mult)
                beta1 = keep1.tile([GMAX, D], f32)
                nc.vector.tensor_tensor(out=beta1[:], in0=mean[:], in1=csb['gn1_ms16'][:], op=AO.mult)
                nc.vector.tensor_tensor(out=beta1[:], in0=beta1[:], in1=alpha1[:], op=AO.mult)
                nc.vector.tensor_tensor(out=beta1[:], in0=csb['gn1_b16'][:], in1=beta1[:], op=AO.subtract)
                a1b1 = keep1.tile([GMAX, 2 * D], f16)
                nc.vector.tensor_copy(out=a1b1[:, :D], in_=alpha1[:])
                nc.vector.tensor_copy(out=a1b1[:, D:], in_=beta1[:])
                # apply (grouped, aligned to allgather pieces)
                SPP = NSW // PIECES
                for p in range(PIECES):
                    piece_stores = []
                    for off in range(0, SPP, 4):
                        n = min(4, SPP - off)
                        s0 = p * SPP + off
                        btp = ps1b.tile([GMAX, 4 * 128], f16, tag='btp')
                        for j in range(n):
                            s = s0 + j
                            nc.tensor.transpose(
                                out=btp[:, j * 128:(j + 1) * 128],
                                in_=Bt[s // 4][:, (s % 4) * GMAX:(s % 4 + 1) * GMAX],
                                identity=ident_f[:])
                        bts = sb1.tile([GMAX, 4 * 128], f16, tag='bts')
                        nc.scalar.copy(out=bts[:, :n * 128], in_=btp[:, :n * 128])
                        ab_ps = ps1b.tile([128, 4 * 2 * D], f32, tag='ab')
                        for j in range(n):
                            nc.tensor.matmul(
                                out=ab_ps[:, j * 2 * D:(j + 1) * 2 * D],
                                lhsT=bts[:, j * 128:(j + 1) * 128], rhs=a1b1[:],
                                start=True, stop=True)
                        hw2 = sb1.tile([128, 4 * D], f16, tag='h_in2')
                        nc.sync.dma_start(
                            out=hw2[:, :n * D].rearrange('p (s d) -> p s d', d=D),
                            in_=h_slice[s0 * 128:(s0 + n) * 128, :]
                                .rearrange('(s p) d -> p s d', p=128))
                        abv = ab_ps[:].rearrange('p (s e) -> p s e', e=2 * D)
                        hnw = sb1.tile([128, 4 * D], f16, tag='hnw')
                        hnwv = hnw[:].rearrange('p (s d) -> p s d', d=D)
                        hwv = hw2[:].rearrange('p (s d) -> p s d', d=D)
                        nc.vector.tensor_tensor(out=hnwv[:, :n, :], in0=hwv[:, :n, :],
                                                in1=abv[:, :n, :D], op=AO.mult)
                        nc.vector.tensor_tensor(out=hnwv[:, :n, :], in0=hnwv[:, :n, :],
                                                in1=abv[:, :n, D:], op=AO.add)
                        stin = nc.sync.dma_start(
                            out=hn_local[s0 * 128:(s0 + n) * 128, :]
                                .rearrange('(s p) d -> p s d', p=128),
                            in_=hnwv[:, :n, :])
                        piece_stores.append(stin)
                    cc = nc.gpsimd.collective_compute(
                        'AllGather', AO.bypass,
                        replica_groups=[list(range(NCORES))],
                        ins=[hn_local[p * RPP:(p + 1) * RPP, :]],
                        outs=[hn_full[p * NCORES * RPP:(p + 1) * NCORES * RPP, :]
                              .rearrange('(c r) d -> c r d', c=NCORES)])
                    for stx in piece_stores:
                        add_dep_helper(cc.ins, stx.ins, True, 'allgather reads hn piece')
                    cc_hn.append(cc)
            nc.leave_named_scope('ph1_gn1', sc1[0], False)

            # =========== phase 2: relconv (fused QKV) ===========
            sc2 = nc.enter_named_scope('ph2_relconv', False)
            cc_kv = []
            with (
                tc.tile_pool(name='p2i', bufs=1) as ip2,
                tc.tile_pool(name='p2g', bufs=4) as gp2,
                tc.tile_pool(name='p2a', bufs=3) as ap2,
                tc.tile_pool(name='p2sb', bufs=3) as sb2,
                tc.tile_pool(name='p2S', bufs=1, space='PSUM') as psS,
                tc.tile_pool(name='p2qkv', bufs=1, space='PSUM') as psQ,
                tc.tile_pool(name='p2tr', bufs=1, space='PSUM') as psT,
            ):
                rci = ip2.tile([128, RC_CHUNKS * 8], i16)
                nc.sync.dma_start(out=rci[:], in_=rc_idx[:])
                piece_stores = []
                for w in range(NW):
                    qkv_ps = [psQ.tile([128, 3 * D], f32, tag=f'qkv{i}', name=f'qkv{i}')
                              for i in range(2)]
                    S_ps = [psS.tile([128, 512 if p < 4 else 256], f32,
                                     tag=f'S{p}', name=f'S{p}') for p in range(NPAIR)]
                    for hh in range(PIECES):
                        base = hh * BASEQ
                        co = (w * PIECES + hh) * RC_TAKE
                        gt = gp2.tile([128, RC_TAKE * D], f16, tag='g')
                        gi = nc.gpsimd.dma_gather(
                            out_ap=gt[:].rearrange('p (c e) -> p c e', e=D),
                            in_ap=hn_full[base:NTAB, :],
                            idxs_ap=rci[:, co * 8:(co + RC_TAKE) * 8],
                            num_idxs=RC_TAKE * 128, num_idxs_reg=RC_TAKE * 128,
                            elem_size=D, queue_num=hh % NQUEUE, single_packet=False)
                        add_dep_helper(gi.ins, cc_hn[hh].ins, True,
                                       'gather reads hn piece')
                        # batched one-hot build for all chunks of this call
                        A2 = ap2.tile([128, RC_TAKE * 512], f16, tag='A2')
                        nc.vector.tensor_tensor(
                            out=A2[:].rearrange('p (c e) -> p c e', e=512),
                            in0=rck[:, co:co + RC_TAKE]
                                .rearrange('p (c o) -> p c o', o=1)
                                .to_broadcast([128, RC_TAKE, 512]),
                            in1=iota[:].rearrange('p (o e) -> p o e', o=1)
                                .to_broadcast([128, RC_TAKE, 512]),
                            op=AO.is_equal)
                        for ci in range(RC_TAKE):
                            p = ci // K_RC
                            wd = 512 if p < 4 else 256
                            nc.tensor.matmul(
                                out=S_ps[p][:], lhsT=gt[:, ci * D:(ci + 1) * D],
                                rhs=A2[:, ci * 512:ci * 512 + wd],
                                start=(hh == 0 and ci % K_RC == 0),
                                stop=(hh == PIECES - 1 and ci % K_RC == K_RC - 1))
                    # copy S to sbuf (fp16)
                    stp = []
                    for p in range(NPAIR):
                        wd = 512 if p < 4 else 256
                        stx = sb2.tile([128, 512], f16, tag=f'St{p}')
                        nc.scalar.copy(out=stx[:, :wd], in_=S_ps[p][:])
                        stp.append(stx)
                    # transforms + self-loop + bias + relu
                    for sub in range(2):
                        row0 = w * WIN + sub * 128
                        for r in range(REL):
                            p, par = r // 2, r % 2
                            lhs = stp[p][:, par * 256 + sub * 128: par * 256 + sub * 128 + 128]
                            nc.tensor.matmul(out=qkv_ps[sub][:], lhsT=lhs,
                                             rhs=wrel_sb[:, r * 3 * D:(r + 1) * 3 * D],
                                             start=(r == 0), stop=False)
                        hnw2 = sb2.tile([128, D], f16, tag='hnl')
                        nc.sync.dma_start(out=hnw2[:], in_=hn_local[row0:row0 + 128, :])
                        ht_ps = psT.tile([128, 128], f16, tag='ht')
                        nc.tensor.transpose(out=ht_ps[:], in_=hnw2[:], identity=ident_f[:])
                        ht = sb2.tile([128, 128], f16, tag='htsb')
                        nc.scalar.copy(out=ht[:], in_=ht_ps[:])
                        nc.tensor.matmul(out=qkv_ps[sub][:], lhsT=ht[:], rhs=wloop_sb[:],
                                         start=False, stop=False)
                        nc.tensor.matmul(out=qkv_ps[sub][:], lhsT=ones_f[:], rhs=brqkv_sb[:],
                                         start=False, stop=True)
                        qk = sb2.tile([128, D], b16, tag='qsb')
                        nc.scalar.activation(out=qk[:], in_=qkv_ps[sub][:, :D], func=AF.Relu)
                        nc.sync.dma_start(out=q_local[row0:row0 + 128, :], in_=qk[:])
                        kvsb = sb2.tile([128, 2 * D], b16, tag='kvsb')
                        nc.scalar.activation(out=kvsb[:], in_=qkv_ps[sub][:, D:], func=AF.Relu)
                        stin = nc.sync.dma_start(out=kv_local[row0:row0 + 128, :], in_=kvsb[:])
                        piece_stores.append(stin)
                    if (w + 1) % (NW // PIECES) == 0:
                        p = (w + 1) // (NW // PIECES) - 1
                        cc = nc.gpsimd.collective_compute(
                            'AllGather', AO.bypass,
                            replica_groups=[list(range(NCORES))],
                            ins=[kv_local[p * RPP:(p + 1) * RPP, :]],
                            outs=[kv_full[p * NCORES * RPP:(p + 1) * NCORES * RPP, :]
                                  .rearrange('(c r) d -> c r d', c=NCORES)])
                        for stx in piece_stores:
                            add_dep_helper(cc.ins, stx.ins, True, 'allgather reads kv piece')
                        piece_stores = []
                        cc_kv.append(cc)
            nc.leave_named_scope('ph2_relconv', sc2[0], False)

            # =========== phase 3: attention ===========
            sc3 = nc.enter_named_scope('ph3_attn', False)
            with (
                tc.tile_pool(name='p3i', bufs=1) as ip3,
                tc.tile_pool(name='p3g', bufs=2) as gp3,
                tc.tile_pool(name='p3sb', bufs=2) as sb3,
                tc.tile_pool(name='p3at', bufs=1, space='PSUM') as psA,
                tc.tile_pool(name='p3qd', bufs=1, space='PSUM') as psQd,
                tc.tile_pool(name='p3wv', bufs=1, space='PSUM') as psW,
                tc.tile_pool(name='p3ep', bufs=1, space='PSUM') as psE,
            ):
                ati = ip3.tile([128, AT_CHUNKS * 8], i16)
                nc.sync.dma_start(out=ati[:], in_=at_idx[:])
                for swb in range(NSWB):
                    stg = gp3.tile([128, PIECES * AT_TAKE * 2 * D], b16, tag='stg')
                    for hh in range(PIECES):
                        base = hh * BASEQ
                        co = (swb * PIECES + hh) * AT_TAKE
                        gi = nc.gpsimd.dma_gather(
                            out_ap=stg[:, hh * AT_TAKE * 2 * D:(hh + 1) * AT_TAKE * 2 * D]
                                .rearrange('p (c e) -> p c e', e=2 * D),
                            in_ap=kv_full[base:NTAB, :],
                            idxs_ap=ati[:, co * 8:(co + AT_TAKE) * 8],
                            num_idxs=AT_TAKE * 128, num_idxs_reg=AT_TAKE * 128,
                            elem_size=2 * D, queue_num=hh % NQUEUE, single_packet=False)
                        add_dep_helper(gi.ins, cc_kv[hh].ins, True,
                                       'gather reads kv piece')
                    # Q rows for the 4 subwindows of this block
                    qswb = sb3.tile([128, 4 * D], b16, tag='qswb')
                    nc.sync.dma_start(
                        out=qswb[:].rearrange('p (s d) -> p s d', d=D),
                        in_=q_local[swb * 512:(swb + 1) * 512, :]
                            .rearrange('(s p) d -> p s d', p=128))
                    # staging view [p, pc, swin, k, elem]
                    stgv = stg[:].rearrange('p (pc s k e) -> p pc s k e',
                                            pc=PIECES, s=4, k=K_AT, e=2 * D)
                    wvT_ps = psW.tile([128, 4 * D], f32, tag='wvT')
                    z_ps = psW.tile([128, 4 * HEADS], f32, tag='z')
                    for swin in range(4):
                        sw = swb * 4 + swin
                        # one-hot A [slot, (cj, dl)] over this swin's chunks
                        A = sb3.tile([128, CH_SW * 128], b16, tag='A')
                        kb = sw * CH_SW
                        nc.vector.tensor_tensor(
                            out=A[:].rearrange('p (c e) -> p c e', e=128),
                            in0=atk[:, kb:kb + CH_SW]
                                .rearrange('p (c o) -> p c o', o=1)
                                .to_broadcast([128, CH_SW, 128]),
                            in1=iota_b[:].rearrange('p (o e) -> p o e', o=1)
                                .to_broadcast([128, CH_SW, 128]),
                            op=AO.is_equal)
                        at_ps = psA.tile([128, CH_SW * 128], b16, tag='at4')
                        for cj in range(CH_SW):
                            nc.tensor.transpose(
                                out=at_ps[:, cj * 128:(cj + 1) * 128],
                                in_=A[:, cj * 128:(cj + 1) * 128],
                                identity=ident_b[:])
                        at_sb = sb3.tile([128, CH_SW * 128], b16, tag='at_sb')
                        nc.scalar.copy(out=at_sb[:], in_=at_ps[:])
                        qd_ps = psQd.tile([128, CH_SW * D], f32, tag='qd')
                        for cj in range(CH_SW):
                            nc.tensor.matmul(
                                out=qd_ps[:, cj * D:(cj + 1) * D],
                                lhsT=at_sb[:, cj * 128:(cj + 1) * 128],
                                rhs=qswb[:, swin * D:(swin + 1) * D],
                                start=True, stop=True)
                        qd_sb = sb3.tile([128, CH_SW * D], b16, tag='qd_sb')
                        nc.scalar.copy(out=qd_sb[:], in_=qd_ps[:])
                        # kq = K[src] * Q[dst] over all chunks of this swin
                        kq = sb3.tile([128, CH_SW * D], f16, tag='kq')
                        nc.vector.tensor_tensor(
                            out=kq[:].rearrange('p (pc k e) -> p pc k e',
                                                pc=PIECES, k=K_AT),
                            in0=stgv[:, :, swin, :, :D],
                            in1=qd_sb[:].rearrange('p (pc k e) -> p pc k e',
                                                   pc=PIECES, k=K_AT),
                            op=AO.mult)
                        sc = sb3.tile([128, CH_SW * HEADS], f32, tag='sc')
                        nc.vector.reduce_sum(
                            out=sc[:],
                            in_=kq[:].rearrange('p (c e) -> p c e', e=DH),
                            axis=mybir.AxisListType.X)
                        nc.vector.tensor_scalar_min(out=sc[:], in0=sc[:], scalar1=10.0)
                        ex = sb3.tile([128, CH_SW * HEADS], b16, tag='ex')
                        nc.scalar.activation(out=ex[:], in_=sc[:], func=AF.Exp)
                        # vse = V * exp(score), per k half
                        vse = sb3.tile([128, CH_SW * D], b16, tag='vse')
                        vsev = vse[:].rearrange('p (pc k h e) -> p pc k h e',
                                                pc=PIECES, k=K_AT, h=HEADS)
                        exv4 = ex[:].rearrange('p (pc k h) -> p pc k h',
                                               pc=PIECES, k=K_AT)
                        stgV = stgv[:, :, swin, :, D:].rearrange(
                            'p pc k (h e) -> p pc k h e', h=HEADS)
                        for k in range(K_AT):
                            nc.vector.tensor_tensor(
                                out=vsev[:, :, k, :, :],
                                in0=stgV[:, :, k, :, :],
                                in1=exv4[:, :, k, :].unsqueeze(3)
                                    .to_broadcast([128, PIECES, HEADS, DH]),
                                op=AO.mult)
                        # accumulate wvT = vse.T @ A and z = A.T @ exp
                        for cj in range(CH_SW):
                            nc.tensor.matmul(
                                out=wvT_ps[:, swin * D:(swin + 1) * D],
                                lhsT=vse[:, cj * D:(cj + 1) * D],
                                rhs=A[:, cj * 128:(cj + 1) * 128],
                                start=(cj == 0), stop=(cj == CH_SW - 1))
                            nc.tensor.matmul(
                                out=z_ps[:, swin * HEADS:(swin + 1) * HEADS],
                                lhsT=A[:, cj * 128:(cj + 1) * 128],
                                rhs=ex[:, cj * HEADS:(cj + 1) * HEADS],
                                start=(cj == 0), stop=(cj == CH_SW - 1))
                    # ---- block epilogue: 4 subwindows at once ----
                    zr = sb3.tile([128, 4 * HEADS], f32, tag='zr')
                    nc.vector.tensor_scalar_add(out=zr[:], in0=z_ps[:], scalar1=1e-6)
                    zrec = sb3.tile([128, 4 * HEADS], b16, tag='zrec')
                    with nc.allow_low_precision(reason='1/z in bf16 is enough'):
                        nc.vector.reciprocal(out=zrec[:], in_=zr[:])
                    zrT_ps = psE.tile([4 * HEADS, 128], b16, tag='zrT')
                    nc.tensor.transpose(out=zrT_ps[:], in_=zrec[:], identity=ident_b[:])
                    zrT = sb3.tile([4 * HEADS, 128], b16, tag='zrTsb')
                    nc.scalar.copy(out=zrT[:], in_=zrT_ps[:])
                    zrep_ps = psE.tile([128, 4 * D], f32, tag='zrep')
                    for s in range(4):
                        nc.tensor.matmul(
                            out=zrep_ps[:, s * D:(s + 1) * D],
                            lhsT=hrep_sb[:, s * D:(s + 1) * D],
                            rhs=zrT[:], start=True, stop=True)
                    wvT_sb = sb3.tile([128, 4 * D], b16, tag='wvT_sb')
                    nc.scalar.copy(out=wvT_sb[:], in_=wvT_ps[:])
                    if debug:
                        nc.sync.dma_start(out=wv_dbg[swb * 128:(swb + 1) * 128, :],
                                          in_=wvT_sb[:])
                        nc.sync.dma_start(out=z_dbg[swb * 128:(swb + 1) * 128, :],
                                          in_=zr[:])
                    attnT = sb3.tile([128, 4 * D], b16, tag='attnT')
                    nc.vector.tensor_tensor(out=attnT[:], in0=wvT_sb[:],
                                            in1=zrep_ps[:], op=AO.mult)
                    if debug:
                        nc.sync.dma_start(out=attnt_dbg[swb * 128:(swb + 1) * 128, :],
                                          in_=attnT[:])
                        nc.sync.dma_start(out=zrt_dbg[swb * 32:(swb + 1) * 32, :],
                                          in_=zrT[:])
                        nc.sync.dma_start(out=zrec_dbg[swb * 128:(swb + 1) * 128, :],
                                          in_=zrec[:])
                    ho_ps = psE.tile([128, 4 * D], f32, tag='ho')
                    for s in range(4):
                        nc.tensor.matmul(out=ho_ps[:, s * D:(s + 1) * D],
                                         lhsT=attnT[:, s * D:(s + 1) * D],
                                         rhs=ow_sb[:], start=True, stop=False)
                        nc.tensor.matmul(out=ho_ps[:, s * D:(s + 1) * D],
                                         lhsT=ones_b[:], rhs=bro_sb[:],
                                         start=False, stop=True)
                    hob = hobp.tile([128, 4 * D], f16, tag=f'hob_{swb}',
                                    name=f'hob_{swb}')
                    nc.scalar.copy(out=hob[:], in_=ho_ps[:])
                    hobt.append(hob)
                    if debug:
                        hov = sb3.tile([128, 4 * D], f32, tag='hov')
                        nc.vector.tensor_copy(out=hov[:], in_=ho_ps[:])
                        nc.sync.dma_start(
                            out=hov_dbg[swb * 512:(swb + 1) * 512, :]
                                .rearrange('(s p) d -> p s d', p=128),
                            in_=hov[:].rearrange('p (s d) -> p s d', d=D))
                    if debug:
                        nc.sync.dma_start(
                            out=hob_dbg[swb * 512:(swb + 1) * 512, :]
                                .rearrange('(s p) d -> p s d', p=128),
                            in_=hob[:].rearrange('p (s d) -> p s d', d=D))
                    nc.vector.tensor_reduce(
                        out=ln1_ns[:, swb * 4:swb * 4 + 4],
                        in_=hob[:].rearrange('p (s d) -> p s d', d=D),
                        axis=mybir.AxisListType.X, op=AO.add, negate=True)
                    hsq2 = sb3.tile([128, 4 * D], f16, tag='hsq2')
                    nc.scalar.activation(out=hsq2[:], in_=hob[:], func=AF.Square)
                    nc.vector.tensor_reduce(
                        out=ln1_sq[:, swb * 4:swb * 4 + 4],
                        in_=hsq2[:].rearrange('p (s d) -> p s d', d=D),
                        axis=mybir.AxisListType.X, op=AO.add)
            nc.leave_named_scope('ph3_attn', sc3[0], False)

            # =========== phase 4: LN1 finalize + gn2 stats ===========
            sc4 = nc.enter_named_scope('ph4_stats', False)
            with (
                tc.tile_pool(name='p4sb', bufs=3) as sb4,
                tc.tile_pool(name='p4ps', bufs=1, space='PSUM') as ps4,
            ):
                nmu = sb4.tile([128, NSW], f32, tag='nmu')
                nc.vector.tensor_scalar_mul(out=nmu[:], in0=ln1_ns[:], scalar1=1.0 / D)
                e2 = sb4.tile([128, NSW], f32, tag='e2')
                nc.vector.tensor_scalar_mul(out=e2[:], in0=ln1_sq[:], scalar1=1.0 / D)
                msq1 = sb4.tile([128, NSW], f32, tag='msq1')
                nc.vector.tensor_tensor(out=msq1[:], in0=nmu[:], in1=nmu[:], op=AO.mult)
                nc.vector.tensor_tensor(out=msq1[:], in0=e2[:], in1=msq1[:], op=AO.subtract)
                nc.vector.tensor_scalar_add(out=msq1[:], in0=msq1[:], scalar1=1e-5)
                rv1 = sb4.tile([128, NSW], f32, tag='rv1')
                nc.vector.reciprocal(out=rv1[:], in_=msq1[:])
                nc.scalar.activation(out=ln1_rsd[:], in_=rv1[:], func=AF.Sqrt)
                nc.vector.tensor_tensor(out=ln1_nmr[:], in0=nmu[:], in1=ln1_rsd[:], op=AO.mult)
                sum2 = ps4.tile([GMAX, D], f32, tag='sum2')
                sq2 = ps4.tile([GMAX, D], f32, tag='sq2')
                for s in range(NSW):
                    hob_s = hobt[s // 4][:, (s % 4) * D:(s % 4 + 1) * D]
                    y = sb4.tile([128, D], f16, tag='y4')
                    nc.vector.tensor_scalar(out=y[:], in0=hob_s,
                                            scalar1=ln1_rsd[:, s:s + 1],
                                            scalar2=ln1_nmr[:, s:s + 1],
                                            op0=AO.mult, op1=AO.add)
                    ysq = sb4.tile([128, D], f16, tag='ysq')
                    nc.scalar.activation(out=ysq[:], in_=y[:], func=AF.Square)
                    B_s = Bt[s // 4][:, (s % 4) * GMAX:(s % 4 + 1) * GMAX]
                    nc.tensor.matmul(out=sum2[:], lhsT=B_s, rhs=y[:],
                                     start=(s == 0), stop=(s == NSW - 1))
                    nc.tensor.matmul(out=sq2[:], lhsT=B_s, rhs=ysq[:],
                                     start=(s == 0), stop=(s == NSW - 1))

                # gn2 finalize with LN1 affine folded in
                S1 = keep4.tile([GMAX, D], f32)
                nc.vector.tensor_tensor(out=S1[:], in0=sum2[:],
                                        in1=invc_sb[:].to_broadcast([GMAX, D]), op=AO.mult)
                S2 = keep4.tile([GMAX, D], f32)
                nc.vector.tensor_tensor(out=S2[:], in0=sq2[:],
                                        in1=invc_sb[:].to_broadcast([GMAX, D]), op=AO.mult)
                g1 = csb['ln1_g16']; b1 = csb['ln1_b16']
                mh = keep4.tile([GMAX, D], f32)
                nc.vector.tensor_tensor(out=mh[:], in0=S1[:], in1=g1[:], op=AO.mult)
                nc.vector.tensor_tensor(out=mh[:], in0=mh[:], in1=b1[:], op=AO.add)
                t1 = keep4.tile([GMAX, D], f32)
                nc.vector.tensor_tensor(out=t1[:], in0=S2[:], in1=g1[:], op=AO.mult)
                nc.vector.tensor_tensor(out=t1[:], in0=t1[:], in1=g1[:], op=AO.mult)
                t2 = keep4.tile([GMAX, D], f32)
                nc.vector.tensor_tensor(out=t2[:], in0=S1[:], in1=g1[:], op=AO.mult)
                nc.vector.tensor_tensor(out=t2[:], in0=t2[:], in1=b1[:], op=AO.mult)
                nc.vector.tensor_scalar_mul(out=t2[:], in0=t2[:], scalar1=2.0)
                nc.vector.tensor_tensor(out=t1[:], in0=t1[:], in1=t2[:], op=AO.add)
                nc.vector.tensor_tensor(out=t2[:], in0=b1[:], in1=b1[:], op=AO.mult)
                nc.vector.tensor_tensor(out=t1[:], in0=t1[:], in1=t2[:], op=AO.add)
                # t1 = E[h1^2]
                msq2 = keep4.tile([GMAX, D], f32)
                nc.vector.tensor_tensor(out=msq2[:], in0=mh[:], in1=mh[:], op=AO.mult)
                nc.vector.tensor_tensor(out=msq2[:], in0=msq2[:], in1=csb['gn2_msfac16'][:], op=AO.mult)
                nc.vector.tensor_tensor(out=t1[:], in0=t1[:], in1=msq2[:], op=AO.subtract)
                nc.vector.tensor_scalar_add(out=t1[:], in0=t1[:], scalar1=1e-6)
                std2 = keep4.tile([GMAX, D], f32)
                nc.scalar.activation(out=std2[:], in_=t1[:], func=AF.Sqrt)
                rstd2 = keep4.tile([GMAX, D], f32)
                nc.vector.reciprocal(out=rstd2[:], in_=std2[:])
                alpha2 = keep4.tile([GMAX, D], f32)
                nc.vector.tensor_tensor(out=alpha2[:], in0=rstd2[:], in1=csb['gn2_w16'][:], op=AO.mult)
                A2 = keep4.tile([GMAX, D], f32)
                nc.vector.tensor_tensor(out=A2[:], in0=alpha2[:], in1=g1[:], op=AO.mult)
                B2 = keep4.tile([GMAX, D], f32)
                nc.vector.tensor_tensor(out=B2[:], in0=mh[:], in1=csb['gn2_ms16'][:], op=AO.mult)
                nc.vector.tensor_tensor(out=B2[:], in0=B2[:], in1=alpha2[:], op=AO.mult)
                nc.vector.tensor_tensor(out=B2[:], in0=csb['gn2_b16'][:], in1=B2[:], op=AO.subtract)
                t3 = keep4.tile([GMAX, D], f32)
                nc.vector.tensor_tensor(out=t3[:], in0=b1[:], in1=alpha2[:], op=AO.mult)
                nc.vector.tensor_tensor(out=B2[:], in0=B2[:], in1=t3[:], op=AO.add)
                a2b2 = keep4.tile([GMAX, 2 * D], f16)
                nc.vector.tensor_copy(out=a2b2[:, :D], in_=A2[:])
                nc.vector.tensor_copy(out=a2b2[:, D:], in_=B2[:])
            nc.leave_named_scope('ph4_stats', sc4[0], False)

            # =========== phase 5: gn2 apply + FFN + LN2 center ===========
            sc5 = nc.enter_named_scope('ph5_ffn', False)
            with (
                tc.tile_pool(name='p5sb', bufs=3) as sb5,
                tc.tile_pool(name='p5ps', bufs=1, space='PSUM') as ps5,
                tc.tile_pool(name='p5ps2', bufs=2, space='PSUM') as ps5b,
            ):
                for s0 in range(0, NSW, 2):
                    btp = ps5b.tile([GMAX, 2 * 128], f16, tag='btp2')
                    for j in range(2):
                        s = s0 + j
                        nc.tensor.transpose(
                            out=btp[:, j * 128:(j + 1) * 128],
                            in_=Bt[s // 4][:, (s % 4) * GMAX:(s % 4 + 1) * GMAX],
                            identity=ident_f[:])
                    bts = sb5.tile([GMAX, 2 * 128], f16, tag='bts2')
                    nc.scalar.copy(out=bts[:], in_=btp[:])
                    ab_ps = ps5b.tile([128, 2 * 2 * D], f32, tag='ab2')
                    for j in range(2):
                        nc.tensor.matmul(out=ab_ps[:, j * 2 * D:(j + 1) * 2 * D],
                                         lhsT=bts[:, j * 128:(j + 1) * 128], rhs=a2b2[:],
                                         start=True, stop=True)
                    y5 = sb5.tile([128, 2 * D], f16, tag='y5')
                    for j in range(2):
                        s = s0 + j
                        nc.vector.tensor_scalar(out=y5[:, j * D:(j + 1) * D],
                                                in0=hobt[s // 4][:, (s % 4) * D:(s % 4 + 1) * D],
                                                scalar1=ln1_rsd[:, s:s + 1],
                                                scalar2=ln1_nmr[:, s:s + 1],
                                                op0=AO.mult, op1=AO.add)
                    h2 = sb5.tile([128, 2 * D], f16, tag='h2')
                    y5v = y5[:].rearrange('p (s d) -> p s d', d=D)
                    h2v = h2[:].rearrange('p (s d) -> p s d', d=D)
                    abv = ab_ps[:].rearrange('p (s e) -> p s e', e=2 * D)
                    nc.vector.tensor_tensor(out=h2v, in0=y5v, in1=abv[:, :, :D], op=AO.mult)
                    nc.vector.tensor_tensor(out=h2v, in0=h2v, in1=abv[:, :, D:], op=AO.add)
                    h2t_ps = ps5.tile([128, 2 * D], f16, tag='h2t')
                    for j in range(2):
                        nc.tensor.transpose(out=h2t_ps[:, j * D:(j + 1) * D],
                                            in_=h2[:, j * D:(j + 1) * D],
                                            identity=ident_f[:])
                    h2tt = sb5.tile([128, 2 * D], f16, tag='h2tsb')
                    nc.scalar.copy(out=h2tt[:], in_=h2t_ps[:])
                    f1_ps = ps5.tile([128, 2 * 2 * D], f32, tag='f1')
                    for j in range(2):
                        nc.tensor.matmul(out=f1_ps[:, j * 2 * D:(j + 1) * 2 * D],
                                         lhsT=h2tt[:, j * D:(j + 1) * D], rhs=ffn1_sb[:],
                                         start=True, stop=False)
                        nc.tensor.matmul(out=f1_ps[:, j * 2 * D:(j + 1) * 2 * D],
                                         lhsT=ones_f[:], rhs=brf1_sb[:],
                                         start=False, stop=True)
                    fr = sb5.tile([128, 2 * 2 * D], f16, tag='fr')
                    nc.scalar.activation(out=fr[:], in_=f1_ps[:], func=AF.Relu)
                    frt_ps = ps5.tile([128, 2 * 2 * D], f16, tag='frt')
                    for j in range(2):
                        nc.tensor.transpose(out=frt_ps[:, j * 2 * D:j * 2 * D + D],
                                            in_=fr[:, j * 2 * D:j * 2 * D + D],
                                            identity=ident_f[:])
                        nc.tensor.transpose(out=frt_ps[:, j * 2 * D + D:(j + 1) * 2 * D],
                                            in_=fr[:, j * 2 * D + D:(j + 1) * 2 * D],
                                            identity=ident_f[:])
                    frt = sb5.tile([128, 2 * 2 * D], f16, tag='frtsb')
                    nc.scalar.copy(out=frt[:], in_=frt_ps[:])
                    h3_ps = ps5.tile([128, 2 * D], f32, tag='h3')
                    for j in range(2):
                        sl = slice(j * D, (j + 1) * D)
                        nc.tensor.matmul(out=h3_ps[:, sl],
                                         lhsT=frt[:, j * 2 * D:j * 2 * D + D],
                                         rhs=ffn2_sb[:, :D], start=True, stop=False)
                        nc.tensor.matmul(out=h3_ps[:, sl],
                                         lhsT=frt[:, j * 2 * D + D:(j + 1) * 2 * D],
                                         rhs=ffn2_sb[:, D:], start=False, stop=False)
                        nc.tensor.matmul(out=h3_ps[:, sl], lhsT=ones_f[:],
                                         rhs=brf2_sb[:], start=False, stop=True)
                    # LN2 center + var columns
                    nscol = sb5.tile([128, 2], f32, tag='nscol')
                    nc.vector.tensor_reduce(out=nscol[:],
                                            in_=h3_ps[:].rearrange('p (s d) -> p s d', d=D),
                                            axis=mybir.AxisListType.X, op=AO.add,
                                            negate=True)
                    nc.vector.tensor_scalar_mul(out=nscol[:], in0=nscol[:], scalar1=1.0 / D)
                    xc2 = xc2p.tile([128, 2 * D], f16, tag=f'xc2_{s0 // 2}',
                                    name=f'xc2_{s0 // 2}')
                    for j in range(2):
                        nc.vector.tensor_scalar(out=xc2[:, j * D:(j + 1) * D],
                                                in0=h3_ps[:, j * D:(j + 1) * D],
                                                scalar1=nscol[:, j:j + 1],
                                                scalar2=None, op0=AO.add)
                    xc2t.append(xc2)
                    sqt = sb5.tile([128, 2 * D], f16, tag='sqt')
                    nc.vector.tensor_tensor(out=sqt[:], in0=xc2[:], in1=xc2[:], op=AO.mult)
                    nc.vector.tensor_reduce(out=ln2_vr[:, s0:s0 + 2],
                                            in_=sqt[:].rearrange('p (s d) -> p s d', d=D),
                                            axis=mybir.AxisListType.X, op=AO.add)
            nc.leave_named_scope('ph5_ffn', sc5[0], False)

            # =========== phase 6: LN2 finalize + output ===========
            sc6 = nc.enter_named_scope('ph6_out', False)
            with tc.tile_pool(name='p6sb', bufs=3) as sb6:
                vr2 = sb6.tile([128, NSW], f32, tag='vr2')
                nc.vector.tensor_scalar(out=vr2[:], in0=ln2_vr[:], scalar1=1.0 / D,
                                        scalar2=1e-5, op0=AO.mult, op1=AO.add)
                rv2 = sb6.tile([128, NSW], f32, tag='rv2')
                nc.vector.reciprocal(out=rv2[:], in_=vr2[:])
                nc.scalar.activation(out=ln2_rsd[:], in_=rv2[:], func=AF.Sqrt)
                for s0 in range(0, NSW, 4):
                    ov = sb6.tile([128, 4 * D], f32, tag='ov')
                    for j in range(4):
                        s = s0 + j
                        nc.vector.tensor_scalar(out=ov[:, j * D:(j + 1) * D],
                                                in0=xc2t[s][:],
                                                scalar1=ln2_rsd[:, s:s + 1],
                                                scalar2=None, op0=AO.mult)
                    ovv = ov[:].rearrange('p (s d) -> p s d', d=D)
                    gb = csb['ln2_g'][:].rearrange('p (o d) -> p o d', o=1)
                    bb = csb['ln2_b'][:].rearrange('p (o d) -> p o d', o=1)
                    nc.vector.tensor_tensor(out=ovv, in0=ovv,
                                            in1=gb.to_broadcast([128, 4, D]), op=AO.mult)
                    nc.vector.tensor_tensor(out=ovv, in0=ovv,
                                            in1=bb.to_broadcast([128, 4, D]), op=AO.add)
                    nc.sync.dma_start(
                        out=out_sl[s0 * 128:(s0 + 4) * 128, :]
                            .rearrange('(s p) d -> p s d', p=128),
                        in_=ovv)
            nc.leave_named_scope('ph6_out', sc6[0], False)

            if debug:
                nc.sync.dma_start(out=hn_dbg[:], in_=hn_local[:])
                nc.sync.dma_start(out=kv_dbg[:], in_=kv_local[:])
                nc.sync.dma_start(out=q_dbg[:], in_=q_local[:])

            k4.__exit__(None, None, None)
            xc2p_cm.__exit__(None, None, None)
            hobp_cm.__exit__(None, None, None)
            bpool_cm.__exit__(None, None, None)

    nc.finalize()
    return nc


def kernel(**inputs) -> np.ndarray:
    _ensure_hooks()
    from concourse.bass_utils import run_bass_kernel_spmd

    static, in_maps, meta = preprocess(inputs)
    key = tuple(sorted((k, str(v)) for k, v in static.items()))
    if key not in _PROGRAM_CACHE:
        _PROGRAM_CACHE[key] = build_program(static)
    nc = _PROGRAM_CACHE[key]

    trace = os.environ.get("KERNEL_TRACE") == "1"
    res = run_bass_kernel_spmd(nc, in_maps, list(range(NCORES)), trace=trace)
    global _LAST_RES, _LAST_META
    _LAST_RES, _LAST_META = res, meta
    if trace and res.exec_time_ns:
        print("HW exec time:", res.exec_time_ns, "ns")
    out = np.zeros((N_NODES, D), np.float32)
    for c in range(NCORES):
        n0, n1 = int(meta['n0'][c]), int(meta['n1'][c])
        out[n0:n1] = res.results[c]['out_slice'][:n1 - n0]
    return out



# revision 55
# speedup vs baseline: 1.0370x; 1.0370x over previous
"""GTLayer (relational graph transformer layer) on 8 Trainium2 NeuronCores.

v2 strategy (see kernel_v1_baseline.py for the original):
- Nodes partitioned across 8 cores in graph-aligned contiguous slices;
  edges live with the core owning dst. Global gather tables laid out in 4
  row-pieces so allgathers can be fired piecewise and overlapped.
- fp16 data lane for relconv (hn table, gathered rows, W_rel/W_loop, S
  sums), bf16 lane for attention (kv table, one-hots, V*exp) — exp(score)
  products can exceed fp16 range. PSUM accumulation is always f32.
- RelConv: edges per (dst-window 256, src-half, rel-PAIR); S psum tiles are
  [128, 512] covering two relations (one-hot column = dl + 256*parity), so
  all 9 relations fit one PSUM residency and gather call regions merge to
  (window, half).
- One-hots built with tensor_scalar(is_equal, scalar1=key column) against
  an iota ramp (2x DVE mode), not tensor_tensor broadcasts.
- Attention: sw-blocks of 4 subwindows share gather calls; chunks
  processed in batches of <=4 with batched vector ops.
- Scalar engine does psum->sbuf copies (Copy), Relu/Exp/Square, arranged
  so activation tables almost never reload. LN1/LN2 sqrt is batched
  (columns collected across subwindows, one Sqrt instruction each).
- Biases applied as K=1 matmuls (ones-row lhsT) accumulating into PSUM.
- LN1 affine folded into graphNorm2's alpha/beta algebra.
"""
import os
import sys
import types
import numpy as np

NCORES = 8
N_NODES = 100000
N_EDGES = 600000
D = 128
REL = 9
NPAIR = 5
NG = 64
HEADS = 8
DH = 16
WIN = 256          # relconv dst window
GMAX = 16          # max graphs per core
PIECES = 4         # allgather pieces; also gather base regions (idx >= 0 always)


def _ensure_hooks():
    if "antenv.axon_hooks" not in sys.modules:
        hooks = types.ModuleType("antenv.axon_hooks")
        h = [None]
        hooks.set_axon_ntff_profile_hook = lambda v: h.__setitem__(0, v)
        hooks.get_axon_ntff_profile_hook = lambda: h[0]
        sys.modules["antenv.axon_hooks"] = hooks
        try:
            from trn_agent_boot.trn_boot import _ntff_profile_via_ctypes
            hooks.set_axon_ntff_profile_hook(
                _ntff_profile_via_ctypes("/opt/axon/libaxon_pjrt.so"))
        except Exception:
            pass


# ----------------------------------------------------------------------------
# Host preprocessing
# ----------------------------------------------------------------------------

def _pack_idx16(idx):
    """int16 index array -> [128, n/16] wrapped+replicated layout."""
    n = len(idx)
    assert n % 16 == 0
    blk = idx.reshape(n // 16, 16).T
    return np.tile(blk, (8, 1)).astype(np.int16)


def _layout_slots(order_edges, idx_vals, key_vals, n_chunks):
    """Place edges into n_chunks*128 slots (full 128 per chunk), pads get
    idx 0, key -1. Returns (idx int32, key f32)."""
    tot = n_chunks * 128
    idx = np.zeros(tot, np.int32)
    key = np.full(tot, -1.0, np.float32)
    ne = len(order_edges)
    assert ne <= tot, (ne, n_chunks)
    idx[:ne] = idx_vals[order_edges]
    key[:ne] = key_vals[order_edges]
    return idx, key


def preprocess(inputs):
    import ml_dtypes
    bf16 = ml_dtypes.bfloat16
    h = np.asarray(inputs['h'], np.float32)
    src = np.asarray(inputs['src']).astype(np.int64)
    dst = np.asarray(inputs['dst']).astype(np.int64)
    et = np.asarray(inputs['etypes']).astype(np.int64)
    seg = np.asarray(inputs['seg']).astype(np.int64)

    # --- graph-aligned node partition ---
    gstart = np.searchsorted(seg, np.arange(NG + 1))
    bounds = [0]
    for c in range(1, NCORES):
        target = c * N_NODES / NCORES
        g = int(np.argmin(np.abs(gstart - target)))
        bounds.append(int(gstart[g]))
    bounds.append(N_NODES)
    n0 = np.array(bounds[:-1]); n1 = np.array(bounds[1:])
    sizes = n1 - n0
    ROUND = max(512, WIN * PIECES)
    P_NODES = int(np.ceil(sizes.max() / ROUND) * ROUND)
    RPP = P_NODES // PIECES
    NW = P_NODES // WIN
    NSW = NW * 2
    assert NSW % 4 == 0
    NSWB = NSW // 4
    NTAB = NCORES * P_NODES
    assert NCORES * RPP <= 32768  # per-piece base region fits int16 idx

    owner = np.searchsorted(n1, np.arange(N_NODES), side='right')
    rloc = np.arange(N_NODES) - n0[owner]
    piece = rloc // RPP
    gpos = piece * (NCORES * RPP) + owner * RPP + (rloc - piece * RPP)

    g0 = np.searchsorted(gstart, n0, side='right') - 1
    counts_g = np.diff(gstart).astype(np.float32)

    BASEQ = NCORES * RPP
    srcp = gpos[src]
    half = srcp // BASEQ          # source piece = gather base region (0..3)
    ecore = owner[dst]
    dst_off = dst - n0[ecore]
    w_e = dst_off // WIN
    pair_e = et // 2
    par_e = et % 2
    key512 = (dst_off % WIN + 256 * par_e).astype(np.float32)
    sw_e = dst_off // 128
    dl128 = (dst_off % 128).astype(np.float32)
    swb_e = sw_e // 4
    swin_e = sw_e % 4
    idx_rel = (srcp - half * BASEQ).astype(np.int32)
    assert idx_rel.min() >= 0 and idx_rel.max() < 32768

    # --- relconv chunk structure: uniform K_RC chunks per (w, pc, pair) ---
    NQ = NW * PIECES * NPAIR
    rkey = (w_e * PIECES + half) * NPAIR + pair_e
    rc_counts = np.zeros((NCORES, NQ), np.int64)
    for c in range(NCORES):
        rc_counts[c] = np.bincount(rkey[ecore == c], minlength=NQ)
    K_RC = int(np.ceil(rc_counts / 128.0).max())
    RC_CHUNKS = NQ * K_RC

    # --- attention chunk structure: uniform K_AT per (swb, pc, swin) ---
    NAQ = NSWB * PIECES * 4
    akey = (swb_e * PIECES + half) * 4 + swin_e
    at_counts = np.zeros((NCORES, NAQ), np.int64)
    for c in range(NCORES):
        at_counts[c] = np.bincount(akey[ecore == c], minlength=NAQ)
    K_AT = int(np.ceil(at_counts / 128.0).max())
    AT_CHUNKS = NAQ * K_AT

    # --- per-core data arrays ---
    in_maps = []
    for c in range(NCORES):
        m = np.nonzero(ecore == c)[0]
        # order by (group, srcp) for gather locality
        order = np.lexsort((srcp[m], rkey[m]))
        es = m[order]
        rk = rkey[m][order]
        run_s = np.searchsorted(rk, np.arange(NQ))
        run_e = np.searchsorted(rk, np.arange(NQ) + 1)
        rc_idx = np.zeros(RC_CHUNKS * 128, np.int32)
        rc_key = np.full(RC_CHUNKS * 128, -1.0, np.float32)
        for q in range(NQ):
            ii, kk = _layout_slots(es[run_s[q]:run_e[q]], idx_rel, key512, K_RC)
            rc_idx[q * K_RC * 128:(q + 1) * K_RC * 128] = ii
            rc_key[q * K_RC * 128:(q + 1) * K_RC * 128] = kk

        aorder = np.lexsort((srcp[m], akey[m]))
        aes = m[aorder]
        ak = akey[m][aorder]
        arun_s = np.searchsorted(ak, np.arange(NAQ))
        arun_e = np.searchsorted(ak, np.arange(NAQ) + 1)
        at_idx = np.zeros(AT_CHUNKS * 128, np.int32)
        at_key = np.full(AT_CHUNKS * 128, -1.0, np.float32)
        for q in range(NAQ):
            ii, kk = _layout_slots(aes[arun_s[q]:arun_e[q]], idx_rel, dl128, K_AT)
            at_idx[q * K_AT * 128:(q + 1) * K_AT * 128] = ii
            at_key[q * K_AT * 128:(q + 1) * K_AT * 128] = kk
        # swin-major key copy for batched per-swin one-hots:
        # global chunk (swb, pc, swin, k) -> swin-major (swb, swin, pc, k)
        at_key_sw = (at_key.reshape(NSWB, PIECES, 4, K_AT, 128)
                     .transpose(0, 2, 1, 3, 4).reshape(AT_CHUNKS * 128))

        hs = np.zeros((P_NODES, D), np.float32)
        hs[:sizes[c]] = h[n0[c]:n1[c]]
        segl = np.full(P_NODES, -1.0, np.float32)
        segl[:sizes[c]] = (seg[n0[c]:n1[c]] - g0[c]).astype(np.float32)
        ginc = np.zeros((GMAX, 1), np.float32)
        ng_c = int(seg[n1[c] - 1] - g0[c]) + 1
        assert ng_c <= GMAX
        ginc[:ng_c, 0] = 1.0 / counts_g[g0[c]:g0[c] + ng_c]

        im = {
            'h_slice': hs.astype(np.float16),
            'seg_col': segl.reshape(NSW, 128).T.astype(np.float16),
            'inv_cnt': ginc,
            'rc_idx': _pack_idx16(rc_idx.astype(np.int16)),
            'rc_key': rc_key.reshape(RC_CHUNKS, 128).T.astype(np.float16),
            'at_idx': _pack_idx16(at_idx.astype(np.int16)),
            'at_key_sw': at_key_sw.reshape(AT_CHUNKS, 128).T.astype(np.float16),
        }
        in_maps.append(im)

    # --- shared weights ---
    def A(x):
        return np.ascontiguousarray(np.asarray(x, np.float32))
    Wrel = np.concatenate([
        np.einsum('rb,bio->rio', A(inputs[f'{nm}_coeff']), A(inputs[f'{nm}_basis']))
        for nm in ('q', 'k', 'v')], axis=2)            # [9, 128, 384]
    Wrel[:, :, :D] *= 0.25  # fold score/sqrt(dh) into Q
    wloop = np.concatenate([A(inputs[f'{nm}_loop']) for nm in ('q', 'k', 'v')], 1)
    wloop[:, :D] *= 0.25
    bqkv = np.concatenate([A(inputs[f'{nm}_bias']) for nm in ('q', 'k', 'v')])
    bqkv[:D] *= 0.25
    ffn2p = np.zeros((D, 2 * D), np.float32)           # two K-chunks side by side
    ffn2p[:, :D] = A(inputs['ffn2_w'])[:D, :]
    ffn2p[:, D:] = A(inputs['ffn2_w'])[D:, :]
    # head-replication one-hots: hrep[(s', h), f] = 1 if s' == s and f // DH == h
    hrep = np.zeros((4 * HEADS, 4 * D), bf16)
    for s in range(4):
        for h in range(HEADS):
            hrep[s * HEADS + h, s * D + h * DH:s * D + (h + 1) * DH] = 1.0
    w_shared = {
        'w_rel': A(Wrel.reshape(REL * D, 3 * D)).astype(np.float16),
        'w_loop': wloop.astype(np.float16),
        'brow_qkv': bqkv.reshape(1, 3 * D).astype(np.float16),
        'o_w': A(inputs['o_w']).astype(bf16),
        'brow_o': A(inputs['o_b']).reshape(1, D).astype(bf16),
        'brow_o4': np.tile(A(inputs['o_b']).reshape(1, D), (1, 4)).astype(bf16),
        'hrep': hrep,
        'ffn1': A(inputs['ffn1_w']).astype(np.float16),
        'brow_f1': A(inputs['ffn1_b']).reshape(1, 2 * D).astype(np.float16),
        'ffn2': ffn2p.astype(np.float16),
        'brow_f2': A(inputs['ffn2_b']).reshape(1, D).astype(np.float16),
        'ln2_g': np.tile(A(inputs['ln2_g'])[None, :], (128, 1)),
        'ln2_b': np.tile(A(inputs['ln2_b'])[None, :], (128, 1)),
    }
    for nm in ('gn1', 'gn2'):
        w = A(inputs[f'{nm}_w']); b = A(inputs[f'{nm}_b']); ms = A(inputs[f'{nm}_ms'])
        w_shared[f'{nm}_w16'] = np.tile(w[None, :], (GMAX, 1))
        w_shared[f'{nm}_b16'] = np.tile(b[None, :], (GMAX, 1))
        w_shared[f'{nm}_ms16'] = np.tile(ms[None, :], (GMAX, 1))
        w_shared[f'{nm}_msfac16'] = np.tile((ms * (2 - ms))[None, :], (GMAX, 1))
    for nm in ('ln1_g', 'ln1_b'):
        w_shared[f'{nm}16'] = np.tile(A(inputs[nm])[None, :], (GMAX, 1))
    for im in in_maps:
        im.update(w_shared)

    static = dict(P_NODES=P_NODES, NW=NW, NSW=NSW, NSWB=NSWB, RPP=RPP,
                  NTAB=NTAB, K_RC=K_RC, K_AT=K_AT,
                  RC_CHUNKS=RC_CHUNKS, AT_CHUNKS=AT_CHUNKS)
    meta = dict(n0=n0, n1=n1, sizes=sizes)
    return static, in_maps, meta


# ----------------------------------------------------------------------------
# Bass program
# ----------------------------------------------------------------------------

_PROGRAM_CACHE = {}


def build_program(st):
    import concourse.bass as bass
    import concourse.bacc as bacc
    import concourse.mybir as mybir
    import concourse.tile as tile
    from concourse.tile import TileContext
    from concourse.masks import make_identity
    from bass_rust import add_dep_helper

    P_NODES = st['P_NODES']; NW = st['NW']; NSW = st['NSW']
    NSWB = st['NSWB']; RPP = st['RPP']; NTAB = st['NTAB']
    BASEQ = NCORES * RPP
    K_RC = st['K_RC']; K_AT = st['K_AT']
    RC_CHUNKS = st['RC_CHUNKS']; AT_CHUNKS = st['AT_CHUNKS']
    RC_TAKE = NPAIR * K_RC          # chunks per relconv gather call (w, pc)
    AT_TAKE = 4 * K_AT              # chunks per attention gather call (swb, pc)
    CH_SW = PIECES * K_AT           # chunks per subwindow
    f32 = mybir.dt.float32
    f16 = mybir.dt.float16
    b16 = mybir.dt.bfloat16
    i16 = mybir.dt.int16
    AO = mybir.AluOpType
    AF = mybir.ActivationFunctionType

    NQUEUE = int(os.environ.get('KERNEL_NQUEUE', '2'))
    nc = bacc.Bacc(num_swdge_queues=NQUEUE) if NQUEUE > 1 else bacc.Bacc()

    # --- I/O ---
    h_slice = nc.declare_dram_parameter('h_slice', [P_NODES, D], f16, isOutput=False)
    seg_col = nc.declare_dram_parameter('seg_col', [128, NSW], f16, isOutput=False)
    inv_cnt = nc.declare_dram_parameter('inv_cnt', [GMAX, 1], f32, isOutput=False)
    rc_idx = nc.declare_dram_parameter('rc_idx', [128, RC_CHUNKS * 8], i16, isOutput=False)
    rc_keyd = nc.declare_dram_parameter('rc_key', [128, RC_CHUNKS], f16, isOutput=False)
    at_idx = nc.declare_dram_parameter('at_idx', [128, AT_CHUNKS * 8], i16, isOutput=False)
    at_keyd = nc.declare_dram_parameter('at_key_sw', [128, AT_CHUNKS], f16, isOutput=False)
    hrep_d = nc.declare_dram_parameter('hrep', [4 * HEADS, 4 * D], b16, isOutput=False)
    brow_o4 = nc.declare_dram_parameter('brow_o4', [1, 4 * D], b16, isOutput=False)
    w_rel = nc.declare_dram_parameter('w_rel', [REL * D, 3 * D], f16, isOutput=False)
    w_loop = nc.declare_dram_parameter('w_loop', [D, 3 * D], f16, isOutput=False)
    brow_qkv = nc.declare_dram_parameter('brow_qkv', [1, 3 * D], f16, isOutput=False)
    o_w = nc.declare_dram_parameter('o_w', [D, D], b16, isOutput=False)
    brow_o = nc.declare_dram_parameter('brow_o', [1, D], b16, isOutput=False)
    ffn1 = nc.declare_dram_parameter('ffn1', [D, 2 * D], f16, isOutput=False)
    brow_f1 = nc.declare_dram_parameter('brow_f1', [1, 2 * D], f16, isOutput=False)
    ffn2 = nc.declare_dram_parameter('ffn2', [D, 2 * D], f16, isOutput=False)
    brow_f2 = nc.declare_dram_parameter('brow_f2', [1, D], f16, isOutput=False)
    cdecl = {}
    for nm in ('ln2_g', 'ln2_b'):
        cdecl[nm] = nc.declare_dram_parameter(nm, [128, D], f32, isOutput=False)
    for nm in ('gn1_w16', 'gn1_b16', 'gn1_ms16', 'gn1_msfac16',
               'gn2_w16', 'gn2_b16', 'gn2_ms16', 'gn2_msfac16',
               'ln1_g16', 'ln1_b16'):
        cdecl[nm] = nc.declare_dram_parameter(nm, [GMAX, D], f32, isOutput=False)
    out_sl = nc.declare_dram_parameter('out_slice', [P_NODES, D], f32, isOutput=True)

    # --- internal DRAM ---
    hn_local = nc.dram_tensor('hn_local', [P_NODES, D], f16)
    q_local = nc.dram_tensor('q_local', [P_NODES, D], b16)
    kv_local = nc.dram_tensor('kv_local', [P_NODES, 2 * D], b16)
    hn_full = nc.dram_tensor('hn_full', [NTAB, D], f16, addr_space='Shared')
    kv_full = nc.dram_tensor('kv_full', [NTAB, 2 * D], b16, addr_space='Shared')
    debug = os.environ.get('KERNEL_DEBUG') == '1'
    if debug:
        hn_dbg = nc.declare_dram_parameter('hn_dbg', [P_NODES, D], f16, isOutput=True)
        kv_dbg = nc.declare_dram_parameter('kv_dbg', [P_NODES, 2 * D], b16, isOutput=True)
        q_dbg = nc.declare_dram_parameter('q_dbg', [P_NODES, D], b16, isOutput=True)
        wv_dbg = nc.declare_dram_parameter('wv_dbg', [NSWB * 128, 4 * D], b16, isOutput=True)
        z_dbg = nc.declare_dram_parameter('z_dbg', [NSWB * 128, 4 * HEADS], f32, isOutput=True)
        hob_dbg = nc.declare_dram_parameter('hob_dbg', [P_NODES, D], f16, isOutput=True)
        attnt_dbg = nc.declare_dram_parameter('attnt_dbg', [NSWB * 128, 4 * D], b16, isOutput=True)
        hov_dbg = nc.declare_dram_parameter('hov_dbg', [P_NODES, D], f32, isOutput=True)
        zrt_dbg = nc.declare_dram_parameter('zrt_dbg', [NSWB * 32, 128], b16, isOutput=True)
        zrec_dbg = nc.declare_dram_parameter('zrec_dbg', [NSWB * 128, 4 * HEADS], b16, isOutput=True)

    with TileContext(nc) as tc:
        with tc.tile_pool(name='const', bufs=1) as cpool:
            iota = cpool.tile([128, 2 * WIN], f16)
            nc.gpsimd.iota(iota[:], pattern=[[1, 2 * WIN]], base=0,
                           channel_multiplier=0, allow_small_or_imprecise_dtypes=True)
            iota_b = cpool.tile([128, 128], b16)
            nc.gpsimd.iota(iota_b[:], pattern=[[1, 128]], base=0,
                           channel_multiplier=0, allow_small_or_imprecise_dtypes=True)
            ident_f = cpool.tile([128, 128], f16)
            make_identity(nc, ident_f[:])
            ident_b = cpool.tile([128, 128], b16)
            make_identity(nc, ident_b[:])
            ones_f = cpool.tile([1, 128], f16)
            nc.gpsimd.memset(ones_f[:], 1.0)
            ones_b = cpool.tile([1, 128], b16)
            nc.gpsimd.memset(ones_b[:], 1.0)

            segs = cpool.tile([128, NSW], f16)
            nc.sync.dma_start(out=segs[:], in_=seg_col[:])
            rck = cpool.tile([128, RC_CHUNKS], f16)
            nc.sync.dma_start(out=rck[:], in_=rc_keyd[:])
            atk = cpool.tile([128, AT_CHUNKS], f16)
            nc.sync.dma_start(out=atk[:], in_=at_keyd[:])

            wrel_sb = cpool.tile([128, REL * 3 * D], f16)
            for r in range(REL):
                nc.sync.dma_start(out=wrel_sb[:, r * 3 * D:(r + 1) * 3 * D],
                                  in_=w_rel[r * D:(r + 1) * D, :])
            wloop_sb = cpool.tile([128, 3 * D], f16)
            nc.sync.dma_start(out=wloop_sb[:], in_=w_loop[:])
            brqkv_sb = cpool.tile([1, 3 * D], f16)
            nc.sync.dma_start(out=brqkv_sb[:], in_=brow_qkv[:])
            ow_sb = cpool.tile([D, D], b16)
            nc.sync.dma_start(out=ow_sb[:], in_=o_w[:])
            bro_sb = cpool.tile([1, D], b16)
            nc.sync.dma_start(out=bro_sb[:], in_=brow_o[:])
            bro4_sb = cpool.tile([1, 4 * D], b16)
            nc.sync.dma_start(out=bro4_sb[:], in_=brow_o4[:])
            hrep_sb = cpool.tile([4 * HEADS, 4 * D], b16)
            nc.sync.dma_start(out=hrep_sb[:], in_=hrep_d[:])
            ffn1_sb = cpool.tile([D, 2 * D], f16)
            nc.sync.dma_start(out=ffn1_sb[:], in_=ffn1[:])
            brf1_sb = cpool.tile([1, 2 * D], f16)
            nc.sync.dma_start(out=brf1_sb[:], in_=brow_f1[:])
            ffn2_sb = cpool.tile([D, 2 * D], f16)
            nc.sync.dma_start(out=ffn2_sb[:], in_=ffn2[:])
            brf2_sb = cpool.tile([1, D], f16)
            nc.sync.dma_start(out=brf2_sb[:], in_=brow_f2[:])
            csb = {}
            for nm, dd in cdecl.items():
                t = cpool.tile(list(dd.shape), f32, tag=f'c_{nm}')
                nc.sync.dma_start(out=t[:], in_=dd[:])
                csb[nm] = t
            invc_sb = cpool.tile([GMAX, 1], f32)
            nc.sync.dma_start(out=invc_sb[:], in_=inv_cnt[:])

            # batched LN stat columns
            ln1_ns = cpool.tile([128, NSW], f32, tag='ln1_ns')
            ln1_sq = cpool.tile([128, NSW], f32, tag='ln1_sq')
            ln1_rsd = cpool.tile([128, NSW], f32, tag='ln1_rsd')
            ln1_nmr = cpool.tile([128, NSW], f32, tag='ln1_nmr')
            ln2_vr = cpool.tile([128, NSW], f32, tag='ln2_vr')
            ln2_rsd = cpool.tile([128, NSW], f32, tag='ln2_rsd')

            # persistent pools
            bpool_cm = tc.tile_pool(name='bpool', bufs=1)
            bpool = bpool_cm.__enter__()
            hobp_cm = tc.tile_pool(name='hobp', bufs=1)
            hobp = hobp_cm.__enter__()
            xc2p_cm = tc.tile_pool(name='xc2p', bufs=1)
            xc2p = xc2p_cm.__enter__()
            k4 = tc.tile_pool(name='p4keep', bufs=1)
            keep4 = k4.__enter__()
            Bt = []      # [128, GMAX] f16 one-hot per subwindow
            hobt = []    # [128, D] f16 attn-out per subwindow
            xc2t = []    # [128, D] f16 LN2-centered per subwindow

            # =========== phase 1: graphNorm1 ===========
            sc1 = nc.enter_named_scope('ph1_gn1', False)
            cc_hn = []
            with (
                tc.tile_pool(name='p1keep', bufs=1) as keep1,
                tc.tile_pool(name='p1sb', bufs=3) as sb1,
                tc.tile_pool(name='p1ps', bufs=1, space='PSUM') as ps1,
                tc.tile_pool(name='p1ps2', bufs=2, space='PSUM') as ps1b,
            ):
                sum_ps = ps1.tile([GMAX, D], f32, tag='sums')
                sq_ps = ps1.tile([GMAX, D], f32, tag='sqs')
                for s0 in range(0, NSW, 4):
                    hw = sb1.tile([128, 4 * D], f16, tag='h_in')
                    nc.sync.dma_start(
                        out=hw[:].rearrange('p (s d) -> p s d', d=D),
                        in_=h_slice[s0 * 128:(s0 + 4) * 128, :]
                            .rearrange('(s p) d -> p s d', p=128))
                    B4 = bpool.tile([128, 4 * GMAX], f16, tag=f'B_{s0 // 4}',
                                    name=f'B_{s0 // 4}')
                    nc.vector.tensor_tensor(
                        out=B4[:].rearrange('p (s g) -> p s g', g=GMAX),
                        in0=segs[:, s0:s0 + 4].rearrange('p (s o) -> p s o', o=1)
                            .to_broadcast([128, 4, GMAX]),
                        in1=iota[:, :GMAX].rearrange('p (o g) -> p o g', o=1)
                            .to_broadcast([128, 4, GMAX]),
                        op=AO.is_equal)
                    Bt.append(B4)
                    hsq = sb1.tile([128, 4 * D], f16, tag='hsq')
                    nc.scalar.activation(out=hsq[:], in_=hw[:], func=AF.Square)
                    for j in range(4):
                        s = s0 + j
                        nc.tensor.matmul(out=sum_ps[:],
                                         lhsT=B4[:, j * GMAX:(j + 1) * GMAX],
                                         rhs=hw[:, j * D:(j + 1) * D],
                                         start=(s == 0), stop=(s == NSW - 1))
                        nc.tensor.matmul(out=sq_ps[:],
                                         lhsT=B4[:, j * GMAX:(j + 1) * GMAX],
                                         rhs=hsq[:, j * D:(j + 1) * D],
                                         start=(s == 0), stop=(s == NSW - 1))
                # finalize -> alpha/beta [GMAX, D] f32
                mean = keep1.tile([GMAX, D], f32)
                nc.vector.tensor_tensor(out=mean[:], in0=sum_ps[:],
                                        in1=invc_sb[:].to_broadcast([GMAX, D]), op=AO.mult)
                ex2 = keep1.tile([GMAX, D], f32)
                nc.vector.tensor_tensor(out=ex2[:], in0=sq_ps[:],
                                        in1=invc_sb[:].to_broadcast([GMAX, D]), op=AO.mult)
                msq = keep1.tile([GMAX, D], f32)
                nc.vector.tensor_tensor(out=msq[:], in0=mean[:], in1=mean[:], op=AO.mult)
                nc.vector.tensor_tensor(out=msq[:], in0=msq[:], in1=csb['gn1_msfac16'][:], op=AO.mult)
                var = keep1.tile([GMAX, D], f32)
                nc.vector.tensor_tensor(out=var[:], in0=ex2[:], in1=msq[:], op=AO.subtract)
                nc.vector.tensor_scalar_add(out=var[:], in0=var[:], scalar1=1e-6)
                std = keep1.tile([GMAX, D], f32)
                nc.scalar.activation(out=std[:], in_=var[:], func=AF.Sqrt)
                rstd = keep1.tile([GMAX, D], f32)
                nc.vector.reciprocal(out=rstd[:], in_=std[:])
                alpha1 = keep1.tile([GMAX, D], f32)
                nc.vector.tensor_tensor(out=alpha1[:], in0=rstd[:], in1=csb['gn1_w16'][:], op=AO.mult)
                beta1 = keep1.tile([GMAX, D], f32)
                nc.vector.tensor_tensor(out=beta1[:], in0=mean[:], in1=csb['gn1_ms16'][:], op=AO.mult)
                nc.vector.tensor_tensor(out=beta1[:], in0=beta1[:], in1=alpha1[:], op=AO.mult)
                nc.vector.tensor_tensor(out=beta1[:], in0=csb['gn1_b16'][:], in1=beta1[:], op=AO.subtract)
                a1b1 = keep1.tile([GMAX, 2 * D], f16)
                nc.vector.tensor_copy(out=a1b1[:, :D], in_=alpha1[:])
                nc.vector.tensor_copy(out=a1b1[:, D:], in_=beta1[:])
                # apply (grouped, aligned to allgather pieces)
                SPP = NSW // PIECES
                for p in range(PIECES):
                    piece_stores = []
                    for off in range(0, SPP, 4):
                        n = min(4, SPP - off)
                        s0 = p * SPP + off
                        btp = ps1b.tile([GMAX, 4 * 128], f16, tag='btp')
                        for j in range(n):
                            s = s0 + j
                            nc.tensor.transpose(
                                out=btp[:, j * 128:(j + 1) * 128],
                                in_=Bt[s // 4][:, (s % 4) * GMAX:(s % 4 + 1) * GMAX],
                                identity=ident_f[:])
                        bts = sb1.tile([GMAX, 4 * 128], f16, tag='bts')
                        nc.scalar.copy(out=bts[:, :n * 128], in_=btp[:, :n * 128])
                        ab_ps = ps1b.tile([128, 4 * 2 * D], f32, tag='ab')
                        for j in range(n):
                            nc.tensor.matmul(
                                out=ab_ps[:, j * 2 * D:(j + 1) * 2 * D],
                                lhsT=bts[:, j * 128:(j + 1) * 128], rhs=a1b1[:],
                                start=True, stop=True)
                        hw2 = sb1.tile([128, 4 * D], f16, tag='h_in2')
                        nc.sync.dma_start(
                            out=hw2[:, :n * D].rearrange('p (s d) -> p s d', d=D),
                            in_=h_slice[s0 * 128:(s0 + n) * 128, :]
                                .rearrange('(s p) d -> p s d', p=128))
                        abv = ab_ps[:].rearrange('p (s e) -> p s e', e=2 * D)
                        hnw = sb1.tile([128, 4 * D], f16, tag='hnw')
                        hnwv = hnw[:].rearrange('p (s d) -> p s d', d=D)
                        hwv = hw2[:].rearrange('p (s d) -> p s d', d=D)
                        nc.vector.tensor_tensor(out=hnwv[:, :n, :], in0=hwv[:, :n, :],
                                                in1=abv[:, :n, :D], op=AO.mult)
                        nc.vector.tensor_tensor(out=hnwv[:, :n, :], in0=hnwv[:, :n, :],
                                                in1=abv[:, :n, D:], op=AO.add)
                        stin = nc.sync.dma_start(
                            out=hn_local[s0 * 128:(s0 + n) * 128, :]
                                .rearrange('(s p) d -> p s d', p=128),
                            in_=hnwv[:, :n, :])
                        piece_stores.append(stin)
                    cc = nc.gpsimd.collective_compute(
                        'AllGather', AO.bypass,
                        replica_groups=[list(range(NCORES))],
                        ins=[hn_local[p * RPP:(p + 1) * RPP, :]],
                        outs=[hn_full[p * NCORES * RPP:(p + 1) * NCORES * RPP, :]
                              .rearrange('(c r) d -> c r d', c=NCORES)])
                    for stx in piece_stores:
                        add_dep_helper(cc.ins, stx.ins, True, 'allgather reads hn piece')
                    cc_hn.append(cc)
            nc.leave_named_scope('ph1_gn1', sc1[0], False)

            # =========== phase 2: relconv (fused QKV) ===========
            sc2 = nc.enter_named_scope('ph2_relconv', False)
            cc_kv = []
            with (
                tc.tile_pool(name='p2i', bufs=1) as ip2,
                tc.tile_pool(name='p2g', bufs=4) as gp2,
                tc.tile_pool(name='p2a', bufs=3) as ap2,
                tc.tile_pool(name='p2sb', bufs=3) as sb2,
                tc.tile_pool(name='p2S', bufs=1, space='PSUM') as psS,
                tc.tile_pool(name='p2qkv', bufs=1, space='PSUM') as psQ,
                tc.tile_pool(name='p2tr', bufs=1, space='PSUM') as psT,
            ):
                rci = ip2.tile([128, RC_CHUNKS * 8], i16)
                nc.sync.dma_start(out=rci[:], in_=rc_idx[:])
                piece_stores = []
                for w in range(NW):
                    qkv_ps = [psQ.tile([128, 3 * D], f32, tag=f'qkv{i}', name=f'qkv{i}')
                              for i in range(2)]
                    S_ps = [psS.tile([128, 512 if p < 4 else 256], f32,
                                     tag=f'S{p}', name=f'S{p}') for p in range(NPAIR)]
                    for hh in range(PIECES):
                        base = hh * BASEQ
                        co = (w * PIECES + hh) * RC_TAKE
                        gt = gp2.tile([128, RC_TAKE * D], f16, tag='g')
                        gi = nc.gpsimd.dma_gather(
                            out_ap=gt[:].rearrange('p (c e) -> p c e', e=D),
                            in_ap=hn_full[base:NTAB, :],
                            idxs_ap=rci[:, co * 8:(co + RC_TAKE) * 8],
                            num_idxs=RC_TAKE * 128, num_idxs_reg=RC_TAKE * 128,
                            elem_size=D, queue_num=hh % NQUEUE)
                        add_dep_helper(gi.ins, cc_hn[hh].ins, True,
                                       'gather reads hn piece')
                        # batched one-hot build for all chunks of this call
                        A2 = ap2.tile([128, RC_TAKE * 512], f16, tag='A2')
                        nc.vector.tensor_tensor(
                            out=A2[:].rearrange('p (c e) -> p c e', e=512),
                            in0=rck[:, co:co + RC_TAKE]
                                .rearrange('p (c o) -> p c o', o=1)
                                .to_broadcast([128, RC_TAKE, 512]),
                            in1=iota[:].rearrange('p (o e) -> p o e', o=1)
                                .to_broadcast([128, RC_TAKE, 512]),
                            op=AO.is_equal)
                        for ci in range(RC_TAKE):
                            p = ci // K_RC
                            wd = 512 if p < 4 else 256
                            nc.tensor.matmul(
                                out=S_ps[p][:], lhsT=gt[:, ci * D:(ci + 1) * D],
                                rhs=A2[:, ci * 512:ci * 512 + wd],
                                start=(hh == 0 and ci % K_RC == 0),
                                stop=(hh == PIECES - 1 and ci % K_RC == K_RC - 1))
                    # copy S to sbuf (fp16)
                    stp = []
                    for p in range(NPAIR):
                        wd = 512 if p < 4 else 256
                        stx = sb2.tile([128, 512], f16, tag=f'St{p}')
                        nc.scalar.copy(out=stx[:, :wd], in_=S_ps[p][:])
                        stp.append(stx)
                    # transforms + self-loop + bias + relu
                    for sub in range(2):
                        row0 = w * WIN + sub * 128
                        for r in range(REL):
                            p, par = r // 2, r % 2
                            lhs = stp[p][:, par * 256 + sub * 128: par * 256 + sub * 128 + 128]
                            nc.tensor.matmul(out=qkv_ps[sub][:], lhsT=lhs,
                                             rhs=wrel_sb[:, r * 3 * D:(r + 1) * 3 * D],
                                             start=(r == 0), stop=False)
                        hnw2 = sb2.tile([128, D], f16, tag='hnl')
                        nc.sync.dma_start(out=hnw2[:], in_=hn_local[row0:row0 + 128, :])
                        ht_ps = psT.tile([128, 128], f16, tag='ht')
                        nc.tensor.transpose(out=ht_ps[:], in_=hnw2[:], identity=ident_f[:])
                        ht = sb2.tile([128, 128], f16, tag='htsb')
                        nc.scalar.copy(out=ht[:], in_=ht_ps[:])
                        nc.tensor.matmul(out=qkv_ps[sub][:], lhsT=ht[:], rhs=wloop_sb[:],
                                         start=False, stop=False)
                        nc.tensor.matmul(out=qkv_ps[sub][:], lhsT=ones_f[:], rhs=brqkv_sb[:],
                                         start=False, stop=True)
                        qk = sb2.tile([128, D], b16, tag='qsb')
                        nc.scalar.activation(out=qk[:], in_=qkv_ps[sub][:, :D], func=AF.Relu)
                        nc.sync.dma_start(out=q_local[row0:row0 + 128, :], in_=qk[:])
                        kvsb = sb2.tile([128, 2 * D], b16, tag='kvsb')
                        nc.scalar.activation(out=kvsb[:], in_=qkv_ps[sub][:, D:], func=AF.Relu)
                        stin = nc.sync.dma_start(out=kv_local[row0:row0 + 128, :], in_=kvsb[:])
                        piece_stores.append(stin)
                    if (w + 1) % (NW // PIECES) == 0:
                        p = (w + 1) // (NW // PIECES) - 1
                        cc = nc.gpsimd.collective_compute(
                            'AllGather', AO.bypass,
                            replica_groups=[list(range(NCORES))],
                            ins=[kv_local[p * RPP:(p + 1) * RPP, :]],
                            outs=[kv_full[p * NCORES * RPP:(p + 1) * NCORES * RPP, :]
                                  .rearrange('(c r) d -> c r d', c=NCORES)])
                        for stx in piece_stores:
                            add_dep_helper(cc.ins, stx.ins, True, 'allgather reads kv piece')
                        piece_stores = []
                        cc_kv.append(cc)
            nc.leave_named_scope('ph2_relconv', sc2[0], False)

            # =========== phase 3: attention ===========
            sc3 = nc.enter_named_scope('ph3_attn', False)
            with (
                tc.tile_pool(name='p3i', bufs=1) as ip3,
                tc.tile_pool(name='p3g', bufs=2) as gp3,
                tc.tile_pool(name='p3sb', bufs=2) as sb3,
                tc.tile_pool(name='p3at', bufs=1, space='PSUM') as psA,
                tc.tile_pool(name='p3qd', bufs=1, space='PSUM') as psQd,
                tc.tile_pool(name='p3wv', bufs=1, space='PSUM') as psW,
                tc.tile_pool(name='p3ep', bufs=1, space='PSUM') as psE,
            ):
                ati = ip3.tile([128, AT_CHUNKS * 8], i16)
                nc.sync.dma_start(out=ati[:], in_=at_idx[:])
                for swb in range(NSWB):
                    stg = gp3.tile([128, PIECES * AT_TAKE * 2 * D], b16, tag='stg')
                    for hh in range(PIECES):
                        base = hh * BASEQ
                        co = (swb * PIECES + hh) * AT_TAKE
                        gi = nc.gpsimd.dma_gather(
                            out_ap=stg[:, hh * AT_TAKE * 2 * D:(hh + 1) * AT_TAKE * 2 * D]
                                .rearrange('p (c e) -> p c e', e=2 * D),
                            in_ap=kv_full[base:NTAB, :],
                            idxs_ap=ati[:, co * 8:(co + AT_TAKE) * 8],
                            num_idxs=AT_TAKE * 128, num_idxs_reg=AT_TAKE * 128,
                            elem_size=2 * D, queue_num=hh % NQUEUE)
                        add_dep_helper(gi.ins, cc_kv[hh].ins, True,
                                       'gather reads kv piece')
                    # Q rows for the 4 subwindows of this block
                    qswb = sb3.tile([128, 4 * D], b16, tag='qswb')
                    nc.sync.dma_start(
                        out=qswb[:].rearrange('p (s d) -> p s d', d=D),
                        in_=q_local[swb * 512:(swb + 1) * 512, :]
                            .rearrange('(s p) d -> p s d', p=128))
                    # staging view [p, pc, swin, k, elem]
                    stgv = stg[:].rearrange('p (pc s k e) -> p pc s k e',
                                            pc=PIECES, s=4, k=K_AT, e=2 * D)
                    wvT_ps = psW.tile([128, 4 * D], f32, tag='wvT')
                    z_ps = psW.tile([128, 4 * HEADS], f32, tag='z')
                    for swin in range(4):
                        sw = swb * 4 + swin
                        # one-hot A [slot, (cj, dl)] over this swin's chunks
                        A = sb3.tile([128, CH_SW * 128], b16, tag='A')
                        kb = sw * CH_SW
                        nc.vector.tensor_tensor(
                            out=A[:].rearrange('p (c e) -> p c e', e=128),
                            in0=atk[:, kb:kb + CH_SW]
                                .rearrange('p (c o) -> p c o', o=1)
                                .to_broadcast([128, CH_SW, 128]),
                            in1=iota_b[:].rearrange('p (o e) -> p o e', o=1)
                                .to_broadcast([128, CH_SW, 128]),
                            op=AO.is_equal)
                        at_ps = psA.tile([128, CH_SW * 128], b16, tag='at4')
                        for cj in range(CH_SW):
                            nc.tensor.transpose(
                                out=at_ps[:, cj * 128:(cj + 1) * 128],
                                in_=A[:, cj * 128:(cj + 1) * 128],
                                identity=ident_b[:])
                        at_sb = sb3.tile([128, CH_SW * 128], b16, tag='at_sb')
                        nc.scalar.copy(out=at_sb[:], in_=at_ps[:])
                        qd_ps = psQd.tile([128, CH_SW * D], f32, tag='qd')
                        for cj in range(CH_SW):
                            nc.tensor.matmul(
                                out=qd_ps[:, cj * D:(cj + 1) * D],
                                lhsT=at_sb[:, cj * 128:(cj + 1) * 128],
                                rhs=qswb[:, swin * D:(swin + 1) * D],
                                start=True, stop=True)
                        qd_sb = sb3.tile([128, CH_SW * D], b16, tag='qd_sb')
                        nc.scalar.copy(out=qd_sb[:], in_=qd_ps[:])
                        # kq = K[src] * Q[dst] over all chunks of this swin
                        kq = sb3.tile([128, CH_SW * D], f16, tag='kq')
                        nc.vector.tensor_tensor(
                            out=kq[:].rearrange('p (pc k e) -> p pc k e',
                                                pc=PIECES, k=K_AT),
                            in0=stgv[:, :, swin, :, :D],
                            in1=qd_sb[:].rearrange('p (pc k e) -> p pc k e',
                                                   pc=PIECES, k=K_AT),
                            op=AO.mult)
                        sc = sb3.tile([128, CH_SW * HEADS], f32, tag='sc')
                        nc.vector.reduce_sum(
                            out=sc[:],
                            in_=kq[:].rearrange('p (c e) -> p c e', e=DH),
                            axis=mybir.AxisListType.X)
                        nc.vector.tensor_scalar_min(out=sc[:], in0=sc[:], scalar1=10.0)
                        ex = sb3.tile([128, CH_SW * HEADS], b16, tag='ex')
                        nc.scalar.activation(out=ex[:], in_=sc[:], func=AF.Exp)
                        # vse = V * exp(score), per k half
                        vse = sb3.tile([128, CH_SW * D], b16, tag='vse')
                        vsev = vse[:].rearrange('p (pc k h e) -> p pc k h e',
                                                pc=PIECES, k=K_AT, h=HEADS)
                        exv4 = ex[:].rearrange('p (pc k h) -> p pc k h',
                                               pc=PIECES, k=K_AT)
                        stgV = stgv[:, :, swin, :, D:].rearrange(
                            'p pc k (h e) -> p pc k h e', h=HEADS)
                        for k in range(K_AT):
                            nc.vector.tensor_tensor(
                                out=vsev[:, :, k, :, :],
                                in0=stgV[:, :, k, :, :],
                                in1=exv4[:, :, k, :].unsqueeze(3)
                                    .to_broadcast([128, PIECES, HEADS, DH]),
                                op=AO.mult)
                        # accumulate wvT = vse.T @ A and z = A.T @ exp
                        for cj in range(CH_SW):
                            nc.tensor.matmul(
                                out=wvT_ps[:, swin * D:(swin + 1) * D],
                                lhsT=vse[:, cj * D:(cj + 1) * D],
                                rhs=A[:, cj * 128:(cj + 1) * 128],
                                start=(cj == 0), stop=(cj == CH_SW - 1))
                            nc.tensor.matmul(
                                out=z_ps[:, swin * HEADS:(swin + 1) * HEADS],
                                lhsT=A[:, cj * 128:(cj + 1) * 128],
                                rhs=ex[:, cj * HEADS:(cj + 1) * HEADS],
                                start=(cj == 0), stop=(cj == CH_SW - 1))
                    # ---- block epilogue: 4 subwindows at once ----
                    zr = sb3.tile([128, 4 * HEADS], f32, tag='zr')
                    nc.vector.tensor_scalar_add(out=zr[:], in0=z_ps[:], scalar1=1e-6)
                    zrec = sb3.tile([128, 4 * HEADS], b16, tag='zrec')
                    with nc.allow_low_precision(reason='1/z in bf16 is enough'):
                        nc.vector.reciprocal(out=zrec[:], in_=zr[:])
                    zrT_ps = psE.tile([4 * HEADS, 128], b16, tag='zrT')
                    nc.tensor.transpose(out=zrT_ps[:], in_=zrec[:], identity=ident_b[:])
                    zrT = sb3.tile([4 * HEADS, 128], b16, tag='zrTsb')
                    nc.scalar.copy(out=zrT[:], in_=zrT_ps[:])
                    zrep_ps = psE.tile([128, 4 * D], f32, tag='zrep')
                    for s in range(4):
                        nc.tensor.matmul(
                            out=zrep_ps[:, s * D:(s + 1) * D],
                            lhsT=hrep_sb[:, s * D:(s + 1) * D],
                            rhs=zrT[:], start=True, stop=True)
                    wvT_sb = sb3.tile([128, 4 * D], b16, tag='wvT_sb')
                    nc.scalar.copy(out=wvT_sb[:], in_=wvT_ps[:])
                    if debug:
                        nc.sync.dma_start(out=wv_dbg[swb * 128:(swb + 1) * 128, :],
                                          in_=wvT_sb[:])
                        nc.sync.dma_start(out=z_dbg[swb * 128:(swb + 1) * 128, :],
                                          in_=zr[:])
                    attnT = sb3.tile([128, 4 * D], b16, tag='attnT')
                    nc.vector.tensor_tensor(out=attnT[:], in0=wvT_sb[:],
                                            in1=zrep_ps[:], op=AO.mult)
                    if debug:
                        nc.sync.dma_start(out=attnt_dbg[swb * 128:(swb + 1) * 128, :],
                                          in_=attnT[:])
                        nc.sync.dma_start(out=zrt_dbg[swb * 32:(swb + 1) * 32, :],
                                          in_=zrT[:])
                        nc.sync.dma_start(out=zrec_dbg[swb * 128:(swb + 1) * 128, :],
                                          in_=zrec[:])
                    ho_ps = psE.tile([128, 4 * D], f32, tag='ho')
                    for s in range(4):
                        nc.tensor.matmul(out=ho_ps[:, s * D:(s + 1) * D],
                                         lhsT=attnT[:, s * D:(s + 1) * D],
                                         rhs=ow_sb[:], start=True, stop=False)
                        nc.tensor.matmul(out=ho_ps[:, s * D:(s + 1) * D],
                                         lhsT=ones_b[:], rhs=bro_sb[:],
                                         start=False, stop=True)
                    hob = hobp.tile([128, 4 * D], f16, tag=f'hob_{swb}',
                                    name=f'hob_{swb}')
                    nc.scalar.copy(out=hob[:], in_=ho_ps[:])
                    hobt.append(hob)
                    if debug:
                        hov = sb3.tile([128, 4 * D], f32, tag='hov')
                        nc.vector.tensor_copy(out=hov[:], in_=ho_ps[:])
                        nc.sync.dma_start(
                            out=hov_dbg[swb * 512:(swb + 1) * 512, :]
                                .rearrange('(s p) d -> p s d', p=128),
                            in_=hov[:].rearrange('p (s d) -> p s d', d=D))
                    if debug:
                        nc.sync.dma_start(
                            out=hob_dbg[swb * 512:(swb + 1) * 512, :]
                                .rearrange('(s p) d -> p s d', p=128),
                            in_=hob[:].rearrange('p (s d) -> p s d', d=D))
                    nc.vector.tensor_reduce(
                        out=ln1_ns[:, swb * 4:swb * 4 + 4],
                        in_=hob[:].rearrange('p (s d) -> p s d', d=D),
                        axis=mybir.AxisListType.X, op=AO.add, negate=True)
                    hsq2 = sb3.tile([128, 4 * D], f16, tag='hsq2')
                    nc.scalar.activation(out=hsq2[:], in_=hob[:], func=AF.Square)
                    nc.vector.tensor_reduce(
                        out=ln1_sq[:, swb * 4:swb * 4 + 4],
                        in_=hsq2[:].rearrange('p (s d) -> p s d', d=D),
                        axis=mybir.AxisListType.X, op=AO.add)
            nc.leave_named_scope('ph3_attn', sc3[0], False)

            # =========== phase 4: LN1 finalize + gn2 stats ===========
            sc4 = nc.enter_named_scope('ph4_stats', False)
            with (
                tc.tile_pool(name='p4sb', bufs=3) as sb4,
                tc.tile_pool(name='p4ps', bufs=1, space='PSUM') as ps4,
            ):
                nmu = sb4.tile([128, NSW], f32, tag='nmu')
                nc.vector.tensor_scalar_mul(out=nmu[:], in0=ln1_ns[:], scalar1=1.0 / D)
                e2 = sb4.tile([128, NSW], f32, tag='e2')
                nc.vector.tensor_scalar_mul(out=e2[:], in0=ln1_sq[:], scalar1=1.0 / D)
                msq1 = sb4.tile([128, NSW], f32, tag='msq1')
                nc.vector.tensor_tensor(out=msq1[:], in0=nmu[:], in1=nmu[:], op=AO.mult)
                nc.vector.tensor_tensor(out=msq1[:], in0=e2[:], in1=msq1[:], op=AO.subtract)
                nc.vector.tensor_scalar_add(out=msq1[:], in0=msq1[:], scalar1=1e-5)
                rv1 = sb4.tile([128, NSW], f32, tag='rv1')
                nc.vector.reciprocal(out=rv1[:], in_=msq1[:])
                nc.scalar.activation(out=ln1_rsd[:], in_=rv1[:], func=AF.Sqrt)
                nc.vector.tensor_tensor(out=ln1_nmr[:], in0=nmu[:], in1=ln1_rsd[:], op=AO.mult)
                sum2 = ps4.tile([GMAX, D], f32, tag='sum2')
                sq2 = ps4.tile([GMAX, D], f32, tag='sq2')
                for s in range(NSW):
                    hob_s = hobt[s // 4][:, (s % 4) * D:(s % 4 + 1) * D]
                    y = sb4.tile([128, D], f16, tag='y4')
                    nc.vector.tensor_scalar(out=y[:], in0=hob_s,
                                            scalar1=ln1_rsd[:, s:s + 1],
                                            scalar2=ln1_nmr[:, s:s + 1],
                                            op0=AO.mult, op1=AO.add)
                    ysq = sb4.tile([128, D], f16, tag='ysq')
                    nc.scalar.activation(out=ysq[:], in_=y[:], func=AF.Square)
                    B_s = Bt[s // 4][:, (s % 4) * GMAX:(s % 4 + 1) * GMAX]
                    nc.tensor.matmul(out=sum2[:], lhsT=B_s, rhs=y[:],
                                     start=(s == 0), stop=(s == NSW - 1))
                    nc.tensor.matmul(out=sq2[:], lhsT=B_s, rhs=ysq[:],
                                     start=(s == 0), stop=(s == NSW - 1))

                # gn2 finalize with LN1 affine folded in
                S1 = keep4.tile([GMAX, D], f32)
                nc.vector.tensor_tensor(out=S1[:], in0=sum2[:],
                                        in1=invc_sb[:].to_broadcast([GMAX, D]), op=AO.mult)
                S2 = keep4.tile([GMAX, D], f32)
                nc.vector.tensor_tensor(out=S2[:], in0=sq2[:],
                                        in1=invc_sb[:].to_broadcast([GMAX, D]), op=AO.mult)
                g1 = csb['ln1_g16']; b1 = csb['ln1_b16']
                mh = keep4.tile([GMAX, D], f32)
                nc.vector.tensor_tensor(out=mh[:], in0=S1[:], in1=g1[:], op=AO.mult)
                nc.vector.tensor_tensor(out=mh[:], in0=mh[:], in1=b1[:], op=AO.add)
                t1 = keep4.tile([GMAX, D], f32)
                nc.vector.tensor_tensor(out=t1[:], in0=S2[:], in1=g1[:], op=AO.mult)
                nc.vector.tensor_tensor(out=t1[:], in0=t1[:], in1=g1[:], op=AO.mult)
                t2 = keep4.tile([GMAX, D], f32)
                nc.vector.tensor_tensor(out=t2[:], in0=S1[:], in1=g1[:], op=AO.mult)
                nc.vector.tensor_tensor(out=t2[:], in0=t2[:], in1=b1[:], op=AO.mult)
                nc.vector.tensor_scalar_mul(out=t2[:], in0=t2[:], scalar1=2.0)
                nc.vector.tensor_tensor(out=t1[:], in0=t1[:], in1=t2[:], op=AO.add)
                nc.vector.tensor_tensor(out=t2[:], in0=b1[:], in1=b1[:], op=AO.mult)
                nc.vector.tensor_tensor(out=t1[:], in0=t1[:], in1=t2[:], op=AO.add)
                # t1 = E[h1^2]
                msq2 = keep4.tile([GMAX, D], f32)
                nc.vector.tensor_tensor(out=msq2[:], in0=mh[:], in1=mh[:], op=AO.mult)
                nc.vector.tensor_tensor(out=msq2[:], in0=msq2[:], in1=csb['gn2_msfac16'][:], op=AO.mult)
                nc.vector.tensor_tensor(out=t1[:], in0=t1[:], in1=msq2[:], op=AO.subtract)
                nc.vector.tensor_scalar_add(out=t1[:], in0=t1[:], scalar1=1e-6)
                std2 = keep4.tile([GMAX, D], f32)
                nc.scalar.activation(out=std2[:], in_=t1[:], func=AF.Sqrt)
                rstd2 = keep4.tile([GMAX, D], f32)
                nc.vector.reciprocal(out=rstd2[:], in_=std2[:])
                alpha2 = keep4.tile([GMAX, D], f32)
                nc.vector.tensor_tensor(out=alpha2[:], in0=rstd2[:], in1=csb['gn2_w16'][:], op=AO.mult)
                A2 = keep4.tile([GMAX, D], f32)
                nc.vector.tensor_tensor(out=A2[:], in0=alpha2[:], in1=g1[:], op=AO.mult)
                B2 = keep4.tile([GMAX, D], f32)
                nc.vector.tensor_tensor(out=B2[:], in0=mh[:], in1=csb['gn2_ms16'][:], op=AO.mult)
                nc.vector.tensor_tensor(out=B2[:], in0=B2[:], in1=alpha2[:], op=AO.mult)
                nc.vector.tensor_tensor(out=B2[:], in0=csb['gn2_b16'][:], in1=B2[:], op=AO.subtract)
                t3 = keep4.tile([GMAX, D], f32)
                nc.vector.tensor_tensor(out=t3[:], in0=b1[:], in1=alpha2[:], op=AO.mult)
                nc.vector.tensor_tensor(out=B2[:], in0=B2[:], in1=t3[:], op=AO.add)
                a2b2 = keep4.tile([GMAX, 2 * D], f16)
                nc.vector.tensor_copy(out=a2b2[:, :D], in_=A2[:])
                nc.vector.tensor_copy(out=a2b2[:, D:], in_=B2[:])
            nc.leave_named_scope('ph4_stats', sc4[0], False)

            # =========== phase 5: gn2 apply + FFN + LN2 center ===========
            sc5 = nc.enter_named_scope('ph5_ffn', False)
            with (
                tc.tile_pool(name='p5sb', bufs=3) as sb5,
                tc.tile_pool(name='p5ps', bufs=1, space='PSUM') as ps5,
                tc.tile_pool(name='p5ps2', bufs=2, space='PSUM') as ps5b,
            ):
                for s0 in range(0, NSW, 2):
                    btp = ps5b.tile([GMAX, 2 * 128], f16, tag='btp2')
                    for j in range(2):
                        s = s0 + j
                        nc.tensor.transpose(
                            out=btp[:, j * 128:(j + 1) * 128],
                            in_=Bt[s // 4][:, (s % 4) * GMAX:(s % 4 + 1) * GMAX],
                            identity=ident_f[:])
                    bts = sb5.tile([GMAX, 2 * 128], f16, tag='bts2')
                    nc.scalar.copy(out=bts[:], in_=btp[:])
                    ab_ps = ps5b.tile([128, 2 * 2 * D], f32, tag='ab2')
                    for j in range(2):
                        nc.tensor.matmul(out=ab_ps[:, j * 2 * D:(j + 1) * 2 * D],
                                         lhsT=bts[:, j * 128:(j + 1) * 128], rhs=a2b2[:],
                                         start=True, stop=True)
                    y5 = sb5.tile([128, 2 * D], f16, tag='y5')
                    for j in range(2):
                        s = s0 + j
                        nc.vector.tensor_scalar(out=y5[:, j * D:(j + 1) * D],
                                                in0=hobt[s // 4][:, (s % 4) * D:(s % 4 + 1) * D],
                                                scalar1=ln1_rsd[:, s:s + 1],
                                                scalar2=ln1_nmr[:, s:s + 1],
                                                op0=AO.mult, op1=AO.add)
                    h2 = sb5.tile([128, 2 * D], f16, tag='h2')
                    y5v = y5[:].rearrange('p (s d) -> p s d', d=D)
                    h2v = h2[:].rearrange('p (s d) -> p s d', d=D)
                    abv = ab_ps[:].rearrange('p (s e) -> p s e', e=2 * D)
                    nc.vector.tensor_tensor(out=h2v, in0=y5v, in1=abv[:, :, :D], op=AO.mult)
                    nc.vector.tensor_tensor(out=h2v, in0=h2v, in1=abv[:, :, D:], op=AO.add)
                    h2t_ps = ps5.tile([128, 2 * D], f16, tag='h2t')
                    for j in range(2):
                        nc.tensor.transpose(out=h2t_ps[:, j * D:(j + 1) * D],
                                            in_=h2[:, j * D:(j + 1) * D],
                                            identity=ident_f[:])
                    h2tt = sb5.tile([128, 2 * D], f16, tag='h2tsb')
                    nc.scalar.copy(out=h2tt[:], in_=h2t_ps[:])
                    f1_ps = ps5.tile([128, 2 * 2 * D], f32, tag='f1')
                    for j in range(2):
                        nc.tensor.matmul(out=f1_ps[:, j * 2 * D:(j + 1) * 2 * D],
                                         lhsT=h2tt[:, j * D:(j + 1) * D], rhs=ffn1_sb[:],
                                         start=True, stop=False)
                        nc.tensor.matmul(out=f1_ps[:, j * 2 * D:(j + 1) * 2 * D],
                                         lhsT=ones_f[:], rhs=brf1_sb[:],
                                         start=False, stop=True)
                    fr = sb5.tile([128, 2 * 2 * D], f16, tag='fr')
                    nc.scalar.activation(out=fr[:], in_=f1_ps[:], func=AF.Relu)
                    frt_ps = ps5.tile([128, 2 * 2 * D], f16, tag='frt')
                    for j in range(2):
                        nc.tensor.transpose(out=frt_ps[:, j * 2 * D:j * 2 * D + D],
                                            in_=fr[:, j * 2 * D:j * 2 * D + D],
                                            identity=ident_f[:])
                        nc.tensor.transpose(out=frt_ps[:, j * 2 * D + D:(j + 1) * 2 * D],
                                            in_=fr[:, j * 2 * D + D:(j + 1) * 2 * D],
                                            identity=ident_f[:])
                    frt = sb5.tile([128, 2 * 2 * D], f16, tag='frtsb')
                    nc.scalar.copy(out=frt[:], in_=frt_ps[:])
                    h3_ps = ps5.tile([128, 2 * D], f32, tag='h3')
                    for j in range(2):
                        sl = slice(j * D, (j + 1) * D)
                        nc.tensor.matmul(out=h3_ps[:, sl],
                                         lhsT=frt[:, j * 2 * D:j * 2 * D + D],
                                         rhs=ffn2_sb[:, :D], start=True, stop=False)
                        nc.tensor.matmul(out=h3_ps[:, sl],
                                         lhsT=frt[:, j * 2 * D + D:(j + 1) * 2 * D],
                                         rhs=ffn2_sb[:, D:], start=False, stop=False)
                        nc.tensor.matmul(out=h3_ps[:, sl], lhsT=ones_f[:],
                                         rhs=brf2_sb[:], start=False, stop=True)
                    # LN2 center + var columns
                    nscol = sb5.tile([128, 2], f32, tag='nscol')
                    nc.vector.tensor_reduce(out=nscol[:],
                                            in_=h3_ps[:].rearrange('p (s d) -> p s d', d=D),
                                            axis=mybir.AxisListType.X, op=AO.add,
                                            negate=True)
                    nc.vector.tensor_scalar_mul(out=nscol[:], in0=nscol[:], scalar1=1.0 / D)
                    xc2 = xc2p.tile([128, 2 * D], f16, tag=f'xc2_{s0 // 2}',
                                    name=f'xc2_{s0 // 2}')
                    for j in range(2):
                        nc.vector.tensor_scalar(out=xc2[:, j * D:(j + 1) * D],
                                                in0=h3_ps[:, j * D:(j + 1) * D],
                                                scalar1=nscol[:, j:j + 1],
                                                scalar2=None, op0=AO.add)
                    xc2t.append(xc2)
                    sqt = sb5.tile([128, 2 * D], f16, tag='sqt')
                    nc.vector.tensor_tensor(out=sqt[:], in0=xc2[:], in1=xc2[:], op=AO.mult)
                    nc.vector.tensor_reduce(out=ln2_vr[:, s0:s0 + 2],
                                            in_=sqt[:].rearrange('p (s d) -> p s d', d=D),
                                            axis=mybir.AxisListType.X, op=AO.add)
            nc.leave_named_scope('ph5_ffn', sc5[0], False)

            # =========== phase 6: LN2 finalize + output ===========
            sc6 = nc.enter_named_scope('ph6_out', False)
            with tc.tile_pool(name='p6sb', bufs=3) as sb6:
                vr2 = sb6.tile([128, NSW], f32, tag='vr2')
                nc.vector.tensor_scalar(out=vr2[:], in0=ln2_vr[:], scalar1=1.0 / D,
                                        scalar2=1e-5, op0=AO.mult, op1=AO.add)
                rv2 = sb6.tile([128, NSW], f32, tag='rv2')
                nc.vector.reciprocal(out=rv2[:], in_=vr2[:])
                nc.scalar.activation(out=ln2_rsd[:], in_=rv2[:], func=AF.Sqrt)
                for s0 in range(0, NSW, 4):
                    ov = sb6.tile([128, 4 * D], f32, tag='ov')
                    for j in range(4):
                        s = s0 + j
                        nc.vector.tensor_scalar(out=ov[:, j * D:(j + 1) * D],
                                                in0=xc2t[s // 2][:, (s % 2) * D:(s % 2 + 1) * D],
                                                scalar1=ln2_rsd[:, s:s + 1],
                                                scalar2=None, op0=AO.mult)
                    ovv = ov[:].rearrange('p (s d) -> p s d', d=D)
                    gb = csb['ln2_g'][:].rearrange('p (o d) -> p o d', o=1)
                    bb = csb['ln2_b'][:].rearrange('p (o d) -> p o d', o=1)
                    nc.vector.tensor_tensor(out=ovv, in0=ovv,
                                            in1=gb.to_broadcast([128, 4, D]), op=AO.mult)
                    nc.vector.tensor_tensor(out=ovv, in0=ovv,
                                            in1=bb.to_broadcast([128, 4, D]), op=AO.add)
                    nc.sync.dma_start(
                        out=out_sl[s0 * 128:(s0 + 4) * 128, :]
                            .rearrange('(s p) d -> p s d', p=128),
                        in_=ovv)
            nc.leave_named_scope('ph6_out', sc6[0], False)

            if debug:
                nc.sync.dma_start(out=hn_dbg[:], in_=hn_local[:])
                nc.sync.dma_start(out=kv_dbg[:], in_=kv_local[:])
                nc.sync.dma_start(out=q_dbg[:], in_=q_local[:])

            k4.__exit__(None, None, None)
            xc2p_cm.__exit__(None, None, None)
            hobp_cm.__exit__(None, None, None)
            bpool_cm.__exit__(None, None, None)

    nc.finalize()
    return nc


def kernel(**inputs) -> np.ndarray:
    _ensure_hooks()
    from concourse.bass_utils import run_bass_kernel_spmd

    static, in_maps, meta = preprocess(inputs)
    key = tuple(sorted((k, str(v)) for k, v in static.items()))
    if key not in _PROGRAM_CACHE:
        _PROGRAM_CACHE[key] = build_program(static)
    nc = _PROGRAM_CACHE[key]

    trace = os.environ.get("KERNEL_TRACE") == "1"
    res = run_bass_kernel_spmd(nc, in_maps, list(range(NCORES)), trace=trace)
    global _LAST_RES, _LAST_META
    _LAST_RES, _LAST_META = res, meta
    if trace and res.exec_time_ns:
        print("HW exec time:", res.exec_time_ns, "ns")
    out = np.zeros((N_NODES, D), np.float32)
    for c in range(NCORES):
        n0, n1 = int(meta['n0'][c]), int(meta['n1'][c])
        out[n0:n1] = res.results[c]['out_slice'][:n1 - n0]
    return out

